# revision 1
# baseline (speedup 1.0000x reference)
"""GQA attention with LoRA-Q, tensor-parallel over 8 TRN2 cores.

Sharding (per core i of 8):
  - Q heads 4i..4i+3 (256 q-dims) and KV head i (GQA: repeat_interleave maps
    q heads [4i,4i+4) exactly onto kv head i).
  - Wq (with LoRA folded: Wq_eff = Wq + lora_B @ lora_A), Wk, Wv row-sharded;
    Wo column-sharded on its input (head) dim.
  - Attention outputs (transposed layout [hd, T]) are AllGathered, then each
    core computes a 256-column slice of the final output.

All matmuls in bf16 with fp32 PSUM accumulation; softmax without max
subtraction (scores are bounded: |S/8| <= ~7), denominator fused into the
PV matmul via an appended ones-column on V.
"""

import numpy as np
import ml_dtypes

import concourse.bass as bass
import concourse.mybir as mybir
import concourse.tile as tile
from concourse import bacc
from concourse.bass_utils import run_bass_kernel_spmd
from concourse.masks import make_identity

BF16 = mybir.dt.bfloat16
F32 = mybir.dt.float32

N_CORES = 8
T = 2048
D = 2048
HD = 64          # head dim
NH = 32          # total q heads
NKV = 8          # total kv heads
NH_LOC = NH // N_CORES       # 4 q heads per core
QW = NH_LOC * HD             # 256 q dims per core
P = 128
KT = D // P                  # 16 contraction tiles
CH = 512                     # T-chunk (psum free dim)
NCH = T // CH                # 4 chunks
NJ = T // P                  # 16 k-blocks
SCALE = 1.0 / 8.0            # 1/sqrt(64)


STOP_AFTER = None  # sim-bisect hook: "proj" | "rope" | "attn" | "norm"


def build_bass(st_group: int = 2):
    nc = bacc.Bacc(None, num_devices=N_CORES)

    # I/O
    xT_d = nc.dram_tensor("xT", [D, T], BF16, kind="ExternalInput")
    w_d = nc.dram_tensor("w_all", [D, QW + 2 * HD], BF16, kind="ExternalInput")
    woT_d = nc.dram_tensor("woT", [D, QW], BF16, kind="ExternalInput")
    cos2_d = nc.dram_tensor("cos2", [P, T], BF16, kind="ExternalInput")
    sin2_d = nc.dram_tensor("sin2", [P, T], BF16, kind="ExternalInput")
    mask_d = nc.dram_tensor("dmask", [P, 4, CH], BF16, kind="ExternalInput")
    y_d = nc.dram_tensor("y", [T, QW], F32, kind="ExternalOutput")

    with tile.TileContext(nc, num_cores=N_CORES) as tc:
        _body(nc, tc, xT_d, w_d, woT_d, cos2_d, sin2_d, mask_d, y_d, st_group)
    nc.compile()
    return nc


def _body(nc, tc, xT_d, w_d, woT_d, cos2_d, sin2_d, mask_d, y_d, st_group):
    import contextlib

    ctx = contextlib.ExitStack()
    with ctx:
        consts = ctx.enter_context(tc.tile_pool(name="consts", bufs=1))
        big = ctx.enter_context(tc.tile_pool(name="big", bufs=1))
        work = ctx.enter_context(tc.tile_pool(name="work", bufs=1))
        rope_p = ctx.enter_context(tc.tile_pool(name="rope_p", bufs=1))
        pt_p = ctx.enter_context(tc.tile_pool(name="pt_p", bufs=3))
        rcp_p = ctx.enter_context(tc.tile_pool(name="rcp_p", bufs=2))
        psum_st = ctx.enter_context(tc.tile_pool(name="psum_st", bufs=2, space="PSUM"))
        psum_o = ctx.enter_context(tc.tile_pool(name="psum_o", bufs=2, space="PSUM"))
        dram = ctx.enter_context(tc.tile_pool(name="dram", bufs=1, space="DRAM"))

        # ---- constants (large loads split per-kt: one dma_start rides a
        # single DMA engine at ~22 GB/s, so chunking is what buys bandwidth)
        w_sb = consts.tile([P, KT, QW + 2 * HD], BF16)
        w_r = w_d.rearrange("(kt p) m -> p kt m", p=P)
        for kt in range(KT):
            nc.sync.dma_start(w_sb[:, kt, :], w_r[:, kt, :])
        woT_sb = consts.tile([P, KT, QW], BF16)
        woT_r = woT_d.rearrange("(kt p) m -> p kt m", p=P)
        for kt in range(KT):
            nc.sync.dma_start(woT_sb[:, kt, :], woT_r[:, kt, :])
        cos2_sb = consts.tile([P, T], BF16)
        nc.sync.dma_start(cos2_sb, cos2_d[:])
        sin2_sb = consts.tile([P, T], BF16)
        nc.sync.dma_start(sin2_sb, sin2_d[:])
        mask_sb = consts.tile([P, 4, CH], BF16)
        nc.sync.dma_start(mask_sb, mask_d[:])
        ident64 = consts.tile([HD, HD], BF16)
        make_identity(nc, ident64)
        ones64 = consts.tile([1, HD], BF16)
        nc.vector.memset(ones64, 1.0)

        # v with ones column appended: [tk(P), j, HD+1]
        v_aug = work.tile([P, NJ, HD + 1], BF16)
        nc.vector.memset(v_aug[:, :, HD : HD + 1], 1.0)

        # ---- load xT resident (32 chunked DMAs across queues)
        xT_sb = big.tile([P, KT, T], BF16, tag="big", name="xT_sb")
        xT_r = xT_d.rearrange("(kt p) t -> p kt t", p=P)
        for kt in range(KT):
            for half in range(2):
                sl = slice(half * (T // 2), (half + 1) * (T // 2))
                nc.sync.dma_start(xT_sb[:, kt, sl], xT_r[:, kt, sl])

        # ---- fused QKV projection (transposed layout): projT[m] rows =
        # [q heads 2m, 2m+1] for m in {0,1}; m=2 rows 0:64 = kT, 64:128 = vT
        projT = work.tile([P, 3, T], BF16)
        for m in range(3):
            for c in range(NCH):
                ps = psum_o.tile([P, CH], F32, tag="mm")
                for kt in range(KT):
                    nc.tensor.matmul(
                        ps,
                        lhsT=w_sb[:, kt, m * P : (m + 1) * P],
                        rhs=xT_sb[:, kt, c * CH : (c + 1) * CH],
                        start=(kt == 0),
                        stop=(kt == KT - 1),
                    )
                nc.vector.tensor_copy(projT[:, m, c * CH : (c + 1) * CH], ps)

        if STOP_AFTER == "proj":
            nc.gpsimd.dma_start(y_d[0:P, :], projT[:, 0, 0:QW])
            return
        # ---- RoPE on q head-pairs -> qT_sb [64, 4, T] (head-major, base 0)
        qT_sb = work.tile([HD, NH_LOC, T], BF16)
        for s in range(2):
            src = projT[:, s, :]
            shuf = rope_p.tile([P, T], BF16, tag="shuf")
            for (a, b) in ((0, 32), (32, 0), (64, 96), (96, 64)):
                nc.sync.dma_start(shuf[a : a + 32, :], src[b : b + 32, :])
            t1 = rope_p.tile([P, T], BF16, tag="t1")
            nc.vector.tensor_mul(t1, src, cos2_sb)
            t2 = rope_p.tile([P, T], BF16, tag="t2")
            nc.vector.tensor_mul(t2, shuf, sin2_sb)
            nc.vector.tensor_add(t1, t1, t2)
            nc.sync.dma_start(qT_sb[:, 2 * s, :], t1[0:HD, :])
            nc.sync.dma_start(qT_sb[:, 2 * s + 1, :], t1[HD:P, :])

        # ---- RoPE on k (rows 0:64 of projT[:,2]) -> kT_sb [64, T]
        kT_sb = work.tile([HD, T], BF16)
        ksrc = projT[0:HD, 2, :]
        kshuf = rope_p.tile([P, T], BF16, tag="shuf", name="kshuf")
        nc.sync.dma_start(kshuf[0:32, :], ksrc[32:HD, :])
        nc.sync.dma_start(kshuf[32:HD, :], ksrc[0:32, :])
        kt1 = rope_p.tile([P, T], BF16, tag="t1", name="kt1")
        nc.vector.tensor_mul(kt1[0:HD, :], ksrc, cos2_sb[0:HD, :])
        kt2 = rope_p.tile([P, T], BF16, tag="t2", name="kt2")
        nc.vector.tensor_mul(kt2[0:HD, :], kshuf[0:HD, :], sin2_sb[0:HD, :])
        nc.vector.tensor_add(kT_sb, kt1[0:HD, :], kt2[0:HD, :])

        # ---- transpose v: vT (projT[64:128, 2]) -> v_aug[:, j, 0:64]
        vT0 = work.tile([HD, T], BF16)
        nc.sync.dma_start(vT0, projT[HD:P, 2, :])
        for j in range(NJ):
            tp = psum_o.tile([P, CH], BF16, tag="mm")
            nc.tensor.transpose(tp[:, 0:HD], vT0[:, j * P : (j + 1) * P], ident64)
            nc.vector.tensor_copy(v_aug[:, j, 0:HD], tp[:, 0:HD])

        if STOP_AFTER == "rope":
            nc.gpsimd.dma_start(y_d[0:HD, :], qT_sb[:, 0, 0:QW])
            return
        # ---- attention per local head, transposed-scores flash style
        # OT_stage rows 0:64 = unnormalized O^T (bf16), row 64 = denominator
        OT_stage = work.tile([HD + 1, NH_LOC, T], BF16)
        OT_sb = work.tile([HD, NH_LOC, T], BF16)
        ot_dram = dram.tile([QW, T], BF16)
        ot_r = ot_dram.rearrange("(h d) t -> d h t", h=NH_LOC)
        G = st_group
        for h in range(NH_LOC):
            for c in range(NCH):
                nj = 4 * c + 4          # causal: k-blocks 0..4c+3
                groups = [
                    list(range(g, min(g + G, nj))) for g in range(0, nj, G)
                ]
                ot = psum_o.tile([P, CH], F32, tag="ot")

                def do_st(js):
                    st = psum_st.tile([P, G, CH], F32, tag="st")
                    for idx, j in enumerate(js):
                        nc.tensor.matmul(
                            st[:, idx, :],
                            lhsT=kT_sb[:, j * P : (j + 1) * P],
                            rhs=qT_sb[:, h, c * CH : (c + 1) * CH],
                            start=True,
                            stop=True,
                        )
                    return st

                def do_rest(st, js):
                    n = len(js)
                    pt = pt_p.tile([P, G, CH], BF16, tag="pt")
                    nc.scalar.activation(
                        pt[:, 0:n, :], st[:, 0:n, :],
                        mybir.ActivationFunctionType.Exp, scale=SCALE,
                    )
                    for idx, j in enumerate(js):
                        if j >= 4 * c:  # diagonal block: zero masked region
                            nc.vector.tensor_mul(
                                pt[:, idx, :], pt[:, idx, :],
                                mask_sb[:, j - 4 * c, :],
                            )
                    for idx, j in enumerate(js):
                        nc.tensor.matmul(
                            ot[0 : HD + 1, :],
                            lhsT=v_aug[:, j, :],
                            rhs=pt[:, idx, :],
                            start=(j == 0),
                            stop=(j == nj - 1),
                            skip_group_check=True,
                        )

                # software-pipeline: issue ST of group g+1 before PV of g
                st_cur = do_st(groups[0])
                for g in range(len(groups)):
                    st_next = do_st(groups[g + 1]) if g + 1 < len(groups) else None
                    do_rest(st_cur, groups[g])
                    st_cur = st_next

                nc.vector.tensor_copy(
                    OT_stage[:, h, c * CH : (c + 1) * CH], ot[0 : HD + 1, :]
                )

            # per-head softmax normalization (overlaps next head's attention)
            den_h = rcp_p.tile([NCH, CH], BF16, tag="den")
            recip_h = rcp_p.tile([NCH, CH], BF16, tag="recip")
            for c in range(NCH):
                nc.sync.dma_start(
                    den_h[c : c + 1, :],
                    OT_stage[HD : HD + 1, h, c * CH : (c + 1) * CH],
                )
            with nc.allow_low_precision("softmax denom in bf16 is fine"):
                nc.vector.reciprocal(recip_h, den_h)
            for c in range(NCH):
                rrow = rcp_p.tile([1, CH], BF16, tag="rrow")
                nc.sync.dma_start(rrow, recip_h[c : c + 1, :])
                bc = psum_o.tile([P, CH], F32, tag="mm")
                nc.tensor.matmul(
                    bc[0:HD, :], lhsT=ones64, rhs=rrow, start=True, stop=True
                )
                nc.vector.tensor_mul(
                    OT_sb[:, h, c * CH : (c + 1) * CH],
                    OT_stage[0:HD, h, c * CH : (c + 1) * CH],
                    bc[0:HD, :],
                )
            nc.sync.dma_start(ot_r[:, h, :], OT_sb[:, h, :])

        if STOP_AFTER == "attn":
            nc.gpsimd.dma_start(y_d[0 : HD + 1, :], OT_stage[:, 0, 0:QW])
            return
        if STOP_AFTER == "norm":
            nc.gpsimd.dma_start(y_d[0:HD, :], OT_sb[:, 0, 0:QW])
            return
        # ---- AllGather of O^T across cores -> [D(=NH*HD), T]
        ofull_dram = dram.tile([D, T], BF16, addr_space="Shared")
        nc.gpsimd.collective_compute(
            "AllGather",
            mybir.AluOpType.bypass,
            replica_groups=[list(range(N_CORES))],
            ins=[ot_dram.opt()],
            outs=[ofull_dram.opt()],
        )

        # ---- final projection: y[:, slice] = O_full @ Wo_slice^T
        ofull_sb = big.tile([P, KT, T], BF16, tag="big", name="ofull_sb")
        of_r = ofull_dram.rearrange("(kt p) t -> p kt t", p=P)
        for kt in range(KT):
            for half in range(2):
                sl = slice(half * (T // 2), (half + 1) * (T // 2))
                nc.sync.dma_start(ofull_sb[:, kt, sl], of_r[:, kt, sl])
        for mt in range(T // P):
            ps = psum_o.tile([P, CH], F32, tag="mm")
            for kt in range(KT):
                nc.tensor.matmul(
                    ps[:, 0:QW],
                    lhsT=ofull_sb[:, kt, mt * P : (mt + 1) * P],
                    rhs=woT_sb[:, kt, :],
                    start=(kt == 0),
                    stop=(kt == KT - 1),
                )
            y_sb = rcp_p.tile([P, QW], F32, tag="y_sb")
            nc.vector.tensor_copy(y_sb, ps[:, 0:QW])
            nc.sync.dma_start(y_d[mt * P : (mt + 1) * P, :], y_sb)


def _prep_shards(x, Wq, lora_A, lora_B, Wk, Wv, Wo):
    bf16 = ml_dtypes.bfloat16
    xT = np.ascontiguousarray(x[0].T).astype(bf16)

    theta = 1.0 / (10000.0 ** (np.arange(0, HD, 2, dtype=np.float32) / HD))
    pos = np.arange(T, dtype=np.float32)
    ang = pos[:, None] * theta[None, :]
    ang = np.concatenate([ang, ang], axis=-1)          # [T, HD]
    cosT = np.cos(ang).T                               # [HD, T]
    sinT = np.sin(ang).T
    sign = np.where(np.arange(HD) < HD // 2, -1.0, 1.0).astype(np.float32)
    sinTs = sinT * sign[:, None]
    cos2 = np.ascontiguousarray(np.concatenate([cosT, cosT], 0)).astype(bf16)
    sin2 = np.ascontiguousarray(np.concatenate([sinTs, sinTs], 0)).astype(bf16)

    p_idx = np.arange(P)[:, None, None]
    m_idx = np.arange(4)[None, :, None]
    f_idx = np.arange(CH)[None, None, :]
    dmask = (p_idx + P * m_idx <= f_idx).astype(bf16)  # [128, 4, 512]

    Wq_eff = Wq + lora_B.astype(np.float64) @ lora_A.astype(np.float64)
    Wq_eff = Wq_eff.astype(np.float32)

    in_maps = []
    for i in range(N_CORES):
        wq_i = Wq_eff[QW * i : QW * (i + 1), :]        # [256, D]
        wk_i = Wk[HD * i : HD * (i + 1), :]            # [64, D]
        wv_i = Wv[HD * i : HD * (i + 1), :]
        w_all = np.ascontiguousarray(
            np.concatenate([wq_i, wk_i, wv_i], 0).T
        ).astype(bf16)                                 # [D, 384]
        woT = np.ascontiguousarray(Wo[QW * i : QW * (i + 1), :].T).astype(bf16)
        in_maps.append({
            "xT": xT,
            "w_all": w_all,
            "woT": woT,
            "cos2": cos2,
            "sin2": sin2,
            "dmask": dmask,
        })
    return in_maps


def run(inputs, trace=False, **kw):
    nc = build_bass()
    in_maps = _prep_shards(**inputs)
    res = run_bass_kernel_spmd(
        nc, in_maps, core_ids=list(range(N_CORES)), trace=trace, **kw
    )
    y = np.concatenate([res.results[i]["y"] for i in range(N_CORES)], axis=1)
    return y[None].astype(np.float32), res


def kernel(**inputs):
    y, _ = run(inputs)
    return y



# revision 16
# speedup vs baseline: 1.7185x; 1.7185x over previous
"""GQA attention with LoRA-Q, tensor-parallel over 8 TRN2 cores.

Sharding (per core i of 8):
  - Q heads 4i..4i+3 (256 q-dims) and KV head i (GQA: repeat_interleave maps
    q heads [4i,4i+4) exactly onto kv head i).
  - Wq (with LoRA folded: Wq_eff = Wq + lora_B @ lora_A), Wk, Wv row-sharded;
    Wo row-parallel on its input (head) dim: each core computes the full-width
    partial y from its own 256 head-dims, then a single ReduceScatter(add)
    leaves each core its 256-row T-slice of y (concatenated on the host).

All matmuls in bf16 with fp32 PSUM accumulation; softmax without max
subtraction (scores are bounded: |S/8| <= ~7), denominator fused into the
PV matmul via an appended ones-column on V.
"""

import numpy as np
import ml_dtypes

import concourse.bass as bass
import concourse.mybir as mybir
import concourse.tile as tile
from concourse import bacc
from concourse.bass_utils import run_bass_kernel_spmd
from concourse.masks import make_identity

BF16 = mybir.dt.bfloat16
F32 = mybir.dt.float32

N_CORES = 8
T = 2048
D = 2048
HD = 64          # head dim
NH = 32          # total q heads
NKV = 8          # total kv heads
NH_LOC = NH // N_CORES       # 4 q heads per core
QW = NH_LOC * HD             # 256 q dims per core
P = 128
KT = D // P                  # 16 contraction tiles
CH = 512                     # T-chunk (psum free dim)
NCH = T // CH                # 4 chunks
NJ = T // P                  # 16 k-blocks
SCALE = 1.0 / 8.0            # 1/sqrt(64)


STOP_AFTER = None  # sim-bisect hook: "proj" | "rope" | "attn" | "norm"


def build_bass(st_group: int = 2):
    nc = bacc.Bacc(None, num_devices=N_CORES)

    # I/O
    xT_d = nc.dram_tensor("xT", [D, T], BF16, kind="ExternalInput")
    w_d = nc.dram_tensor("w_all", [D, QW + 2 * HD], BF16, kind="ExternalInput")
    woT_d = nc.dram_tensor("woT", [QW, D], BF16, kind="ExternalInput")
    cos2_d = nc.dram_tensor("cos2", [P, T], BF16, kind="ExternalInput")
    sin2_d = nc.dram_tensor("sin2", [P, T], BF16, kind="ExternalInput")
    mask_d = nc.dram_tensor("dmask", [P, 4, CH], BF16, kind="ExternalInput")
    y_d = nc.dram_tensor("y", [T // N_CORES, D], BF16, kind="ExternalOutput")

    with tile.TileContext(nc, num_cores=N_CORES) as tc:
        _body(nc, tc, xT_d, w_d, woT_d, cos2_d, sin2_d, mask_d, y_d, st_group)
    nc.compile()
    return nc


def _body(nc, tc, xT_d, w_d, woT_d, cos2_d, sin2_d, mask_d, y_d, st_group):
    import contextlib

    ctx = contextlib.ExitStack()
    with ctx:
        consts = ctx.enter_context(tc.tile_pool(name="consts", bufs=1))
        big = ctx.enter_context(tc.tile_pool(name="big", bufs=1))
        work = ctx.enter_context(tc.tile_pool(name="work", bufs=1))
        rope_p = ctx.enter_context(tc.tile_pool(name="rope_p", bufs=1))
        pt_p = ctx.enter_context(tc.tile_pool(name="pt_p", bufs=3))
        rcp_p = ctx.enter_context(tc.tile_pool(name="rcp_p", bufs=2))
        psum_st = ctx.enter_context(tc.tile_pool(name="psum_st", bufs=2, space="PSUM"))
        psum_o = ctx.enter_context(tc.tile_pool(name="psum_o", bufs=2, space="PSUM"))
        dram = ctx.enter_context(tc.tile_pool(name="dram", bufs=1, space="DRAM"))

        # ---- constants (large loads split per-kt: one dma_start rides a
        # single DMA engine at ~22 GB/s, so chunking is what buys bandwidth)
        w_sb = consts.tile([P, KT, QW + 2 * HD], BF16)
        w_r = w_d.rearrange("(kt p) m -> p kt m", p=P)
        for kt in range(KT):
            nc.sync.dma_start(w_sb[:, kt, :], w_r[:, kt, :])
        # Wo^T rows for the local 256 head-dims: [128, 2, D]
        woT2_sb = consts.tile([P, 2, D], BF16)
        woT2_r = woT_d.rearrange("(g p) o -> p g o", p=P)
        for g in range(2):
            for half in range(2):
                sl = slice(half * (D // 2), (half + 1) * (D // 2))
                nc.sync.dma_start(woT2_sb[:, g, sl], woT2_r[:, g, sl])
        cos2_sb = consts.tile([P, T], BF16)
        nc.sync.dma_start(cos2_sb, cos2_d[:])
        sin2_sb = consts.tile([P, T], BF16)
        nc.sync.dma_start(sin2_sb, sin2_d[:])
        mask_sb = consts.tile([P, 4, CH], BF16)
        nc.sync.dma_start(mask_sb, mask_d[:])
        ident64 = consts.tile([HD, HD], BF16)
        make_identity(nc, ident64)
        ones64 = consts.tile([1, HD], BF16)
        nc.vector.memset(ones64, 1.0)

        # v with ones column appended: [tk(P), j, HD+1]
        v_aug = work.tile([P, NJ, HD + 1], BF16)
        nc.vector.memset(v_aug[:, :, HD : HD + 1], 1.0)

        # ---- load xT resident (32 chunked DMAs across queues), t-half-major
        # so proj chunks c=0,1 can start while the upper half still loads
        xT_sb = big.tile([P, KT, T], BF16, tag="big", name="xT_sb")
        xT_r = xT_d.rearrange("(kt p) t -> p kt t", p=P)
        for half in range(2):
            sl = slice(half * (T // 2), (half + 1) * (T // 2))
            for kt in range(KT):
                nc.sync.dma_start(xT_sb[:, kt, sl], xT_r[:, kt, sl])

        # ---- fused QKV projection (transposed layout): projT[m] rows =
        # [q heads 2m, 2m+1] for m in {0,1}; m=2 rows 0:64 = kT, 64:128 = vT
        projT = work.tile([P, 3, T], BF16)
        for m in range(3):
            for c in range(NCH):
                ps = psum_o.tile([P, CH], F32, tag="mm")
                for kt in range(KT):
                    nc.tensor.matmul(
                        ps,
                        lhsT=w_sb[:, kt, m * P : (m + 1) * P],
                        rhs=xT_sb[:, kt, c * CH : (c + 1) * CH],
                        start=(kt == 0),
                        stop=(kt == KT - 1),
                    )
                nc.vector.tensor_copy(projT[:, m, c * CH : (c + 1) * CH], ps)

        if STOP_AFTER == "proj":
            nc.gpsimd.dma_start(y_d[0:P, 0:QW], projT[:, 0, 0:QW])
            return
        # ---- RoPE on q head-pairs -> qT_sb [64, 4, T] (head-major, base 0)
        qT_sb = work.tile([HD, NH_LOC, T], BF16)
        for s in range(2):
            src = projT[:, s, :]
            shuf = rope_p.tile([P, T], BF16, tag="shuf")
            for (a, b) in ((0, 32), (32, 0), (64, 96), (96, 64)):
                nc.sync.dma_start(shuf[a : a + 32, :], src[b : b + 32, :])
            t1 = rope_p.tile([P, T], BF16, tag="t1")
            nc.vector.tensor_mul(t1, src, cos2_sb)
            t2 = rope_p.tile([P, T], BF16, tag="t2")
            nc.vector.tensor_mul(t2, shuf, sin2_sb)
            nc.vector.tensor_add(t1, t1, t2)
            nc.sync.dma_start(qT_sb[:, 2 * s, :], t1[0:HD, :])
            nc.sync.dma_start(qT_sb[:, 2 * s + 1, :], t1[HD:P, :])

        # ---- RoPE on k (rows 0:64 of projT[:,2]) -> kT_sb [64, T]
        kT_sb = work.tile([HD, T], BF16)
        ksrc = projT[0:HD, 2, :]
        kshuf = rope_p.tile([P, T], BF16, tag="shuf", name="kshuf")
        nc.sync.dma_start(kshuf[0:32, :], ksrc[32:HD, :])
        nc.sync.dma_start(kshuf[32:HD, :], ksrc[0:32, :])
        kt1 = rope_p.tile([P, T], BF16, tag="t1", name="kt1")
        nc.vector.tensor_mul(kt1[0:HD, :], ksrc, cos2_sb[0:HD, :])
        kt2 = rope_p.tile([P, T], BF16, tag="t2", name="kt2")
        nc.vector.tensor_mul(kt2[0:HD, :], kshuf[0:HD, :], sin2_sb[0:HD, :])
        nc.vector.tensor_add(kT_sb, kt1[0:HD, :], kt2[0:HD, :])

        # ---- transpose v: vT (projT[64:128, 2]) -> v_aug[:, j, 0:64]
        vT0 = work.tile([HD, T], BF16)
        nc.sync.dma_start(vT0, projT[HD:P, 2, :])
        for j in range(NJ):
            tp = psum_o.tile([P, CH], BF16, tag="mm")
            nc.tensor.transpose(tp[:, 0:HD], vT0[:, j * P : (j + 1) * P], ident64)
            nc.vector.tensor_copy(v_aug[:, j, 0:HD], tp[:, 0:HD])

        if STOP_AFTER == "rope":
            nc.gpsimd.dma_start(y_d[0:HD, 0:QW], qT_sb[:, 0, 0:QW])
            return
        # ---- attention per local head, transposed-scores flash style
        # OT_stage rows 0:64 = unnormalized O^T (bf16), row 64 = denominator
        OT_stage = work.tile([HD + 1, NH_LOC, T], BF16)
        OT_sb = work.tile([HD, NH_LOC, T], BF16)
        # O^T repacked to 128 partitions: OT128[64*(h%2)+d, h//2, t]
        # (partition p of group g is local head-dim 128*g+p, matching woT2)
        OT128 = work.tile([P, 2, T], BF16)
        G = st_group
        for h in range(NH_LOC):
            for c in range(NCH):
                nj = 4 * c + 4          # causal: k-blocks 0..4c+3
                groups = [
                    list(range(g, min(g + G, nj))) for g in range(0, nj, G)
                ]
                ot = psum_o.tile([P, CH], F32, tag="ot")

                def do_st(js):
                    st = psum_st.tile([P, G, CH], F32, tag="st")
                    for idx, j in enumerate(js):
                        nc.tensor.matmul(
                            st[:, idx, :],
                            lhsT=kT_sb[:, j * P : (j + 1) * P],
                            rhs=qT_sb[:, h, c * CH : (c + 1) * CH],
                            start=True,
                            stop=True,
                        )
                    return st

                def do_rest(st, js):
                    n = len(js)
                    pt = pt_p.tile([P, G, CH], BF16, tag="pt")
                    nc.scalar.activation(
                        pt[:, 0:n, :], st[:, 0:n, :],
                        mybir.ActivationFunctionType.Exp, scale=SCALE,
                    )
                    for idx, j in enumerate(js):
                        if j >= 4 * c:  # diagonal block: zero masked region
                            nc.vector.tensor_mul(
                                pt[:, idx, :], pt[:, idx, :],
                                mask_sb[:, j - 4 * c, :],
                            )
                    for idx, j in enumerate(js):
                        nc.tensor.matmul(
                            ot[0 : HD + 1, :],
                            lhsT=v_aug[:, j, :],
                            rhs=pt[:, idx, :],
                            start=(j == 0),
                            stop=(j == nj - 1),
                            skip_group_check=True,
                        )

                # software-pipeline: issue ST of group g+1 before PV of g
                st_cur = do_st(groups[0])
                for g in range(len(groups)):
                    st_next = do_st(groups[g + 1]) if g + 1 < len(groups) else None
                    do_rest(st_cur, groups[g])
                    st_cur = st_next

                nc.vector.tensor_copy(
                    OT_stage[:, h, c * CH : (c + 1) * CH], ot[0 : HD + 1, :]
                )

            # per-head softmax normalization (overlaps next head's attention)
            den_h = rcp_p.tile([NCH, CH], BF16, tag="den")
            recip_h = rcp_p.tile([NCH, CH], BF16, tag="recip")
            for c in range(NCH):
                nc.sync.dma_start(
                    den_h[c : c + 1, :],
                    OT_stage[HD : HD + 1, h, c * CH : (c + 1) * CH],
                )
            with nc.allow_low_precision("softmax denom in bf16 is fine"):
                nc.vector.reciprocal(recip_h, den_h)
            for c in range(NCH):
                rrow = rcp_p.tile([1, CH], BF16, tag="rrow")
                nc.sync.dma_start(rrow, recip_h[c : c + 1, :])
                bc = psum_o.tile([P, CH], F32, tag="mm")
                nc.tensor.matmul(
                    bc[0:HD, :], lhsT=ones64, rhs=rrow, start=True, stop=True
                )
                nc.vector.tensor_mul(
                    OT_sb[:, h, c * CH : (c + 1) * CH],
                    OT_stage[0:HD, h, c * CH : (c + 1) * CH],
                    bc[0:HD, :],
                )
            h0 = HD * (h % 2)
            nc.sync.dma_start(OT128[h0 : h0 + HD, h // 2, :], OT_sb[:, h, :])

        if STOP_AFTER == "attn":
            nc.gpsimd.dma_start(y_d[0 : HD + 1, 0:QW], OT_stage[:, 0, 0:QW])
            return
        if STOP_AFTER == "norm":
            nc.gpsimd.dma_start(y_d[0:HD, 0:QW], OT_sb[:, 0, 0:QW])
            return
        # ---- partial output projection: part[t, o] = O_local[t, :] @ Wo_local
        # (contraction over the local 256 head-dims), then ReduceScatter(add)
        # over T leaves this core's 256-row slice of y (bf16; host upcasts).
        assert G == 2
        part_dram = dram.tile([T, D], BF16)
        # bf16 staging in SBUF (PSUM can't feed DMA); reuses xT's memory
        pstage = big.tile([P, KT, T], BF16, tag="big", name="pstage")
        cp_engines = [nc.vector, nc.scalar]  # gpsimd cannot read PSUM
        for tt in range(T // P):
            for half in range(2):
                ps = psum_st.tile([P, G, CH], F32, tag="st")
                for oc in range(2):
                    osl = slice((2 * half + oc) * CH, (2 * half + oc + 1) * CH)
                    for g in range(2):
                        nc.tensor.matmul(
                            ps[:, oc, :],
                            lhsT=OT128[:, g, tt * P : (tt + 1) * P],
                            rhs=woT2_sb[:, g, osl],
                            start=(g == 0),
                            stop=(g == 1),
                        )
                eng = cp_engines[(2 * tt + half) % 2]
                dst = pstage[:, tt, half * 2 * CH : (half + 1) * 2 * CH]
                if eng is nc.scalar:
                    nc.scalar.copy(dst, ps[:, 0:2, :])
                else:
                    eng.tensor_copy(dst, ps[:, 0:2, :])
            nc.sync.dma_start(
                part_dram[tt * P : (tt + 1) * P, :], pstage[:, tt, :]
            )
        ys_dram = dram.tile([T // N_CORES, D], BF16)
        nc.gpsimd.collective_compute(
            "ReduceScatter",
            mybir.AluOpType.add,
            replica_groups=[list(range(N_CORES))],
            ins=[part_dram.opt()],
            outs=[ys_dram.opt()],
        )
        for r in range(2):
            nc.sync.dma_start(y_d[r * P : (r + 1) * P, :], ys_dram[r * P : (r + 1) * P, :])


def _prep_shards(x, Wq, lora_A, lora_B, Wk, Wv, Wo):
    bf16 = ml_dtypes.bfloat16
    xT = np.ascontiguousarray(x[0].T).astype(bf16)

    theta = 1.0 / (10000.0 ** (np.arange(0, HD, 2, dtype=np.float32) / HD))
    pos = np.arange(T, dtype=np.float32)
    ang = pos[:, None] * theta[None, :]
    ang = np.concatenate([ang, ang], axis=-1)          # [T, HD]
    cosT = np.cos(ang).T                               # [HD, T]
    sinT = np.sin(ang).T
    sign = np.where(np.arange(HD) < HD // 2, -1.0, 1.0).astype(np.float32)
    sinTs = sinT * sign[:, None]
    cos2 = np.ascontiguousarray(np.concatenate([cosT, cosT], 0)).astype(bf16)
    sin2 = np.ascontiguousarray(np.concatenate([sinTs, sinTs], 0)).astype(bf16)

    p_idx = np.arange(P)[:, None, None]
    m_idx = np.arange(4)[None, :, None]
    f_idx = np.arange(CH)[None, None, :]
    dmask = (p_idx + P * m_idx <= f_idx).astype(bf16)  # [128, 4, 512]

    Wq_eff = Wq + lora_B.astype(np.float64) @ lora_A.astype(np.float64)
    Wq_eff = Wq_eff.astype(np.float32)

    in_maps = []
    for i in range(N_CORES):
        wq_i = Wq_eff[QW * i : QW * (i + 1), :]        # [256, D]
        wk_i = Wk[HD * i : HD * (i + 1), :]            # [64, D]
        wv_i = Wv[HD * i : HD * (i + 1), :]
        w_all = np.ascontiguousarray(
            np.concatenate([wq_i, wk_i, wv_i], 0).T
        ).astype(bf16)                                 # [D, 384]
        # Wo^T rows for this core's head-dims: [256, D]
        woT = np.ascontiguousarray(Wo[:, QW * i : QW * (i + 1)].T).astype(bf16)
        in_maps.append({
            "xT": xT,
            "w_all": w_all,
            "woT": woT,
            "cos2": cos2,
            "sin2": sin2,
            "dmask": dmask,
        })
    return in_maps


def run(inputs, trace=False, **kw):
    nc = build_bass()
    in_maps = _prep_shards(**inputs)
    res = run_bass_kernel_spmd(
        nc, in_maps, core_ids=list(range(N_CORES)), trace=trace, **kw
    )
    y = np.concatenate([res.results[i]["y"] for i in range(N_CORES)], axis=0)
    return y[None].astype(np.float32), res


def kernel(**inputs):
    y, _ = run(inputs)
    return y



# revision 30
# speedup vs baseline: 1.7569x; 1.0224x over previous
"""GQA attention with LoRA-Q, tensor-parallel over 8 TRN2 cores.

Sharding (per core i of 8):
  - Q heads 4i..4i+3 (256 q-dims) and KV head i (GQA: repeat_interleave maps
    q heads [4i,4i+4) exactly onto kv head i).
  - Wq (with LoRA folded: Wq_eff = Wq + lora_B @ lora_A), Wk, Wv row-sharded;
    Wo row-parallel on its input (head) dim: each core computes the full-width
    partial y^T from its own 256 head-dims; four per-T-chunk ReduceScatter(add)
    ops (overlapped with attention of later chunks) leave each core its
    256-feature slice of y^T (transposed + concatenated on the host).

Structure is a single fused c-outer pipeline over the four 512-token chunks:
  proj c -> rope c -> attention (head pairs, shared kv) c -> norm c ->
  partial-Wo drain c -> ReduceScatter #c
so the collective and DMA traffic hide under attention of later chunks.

All matmuls in bf16 with fp32 PSUM accumulation; softmax without max
subtraction (scores are bounded: |S/8| <= ~7), denominator fused into the
PV matmul via an appended ones-column on V. RoPE's rotate-half is a signed
permutation matmul on PE (no partition-shuffle DMAs).
"""

import numpy as np
import ml_dtypes

import concourse.bass as bass
import concourse.mybir as mybir
import concourse.tile as tile
from concourse import bacc
from concourse.bass_utils import run_bass_kernel_spmd

BF16 = mybir.dt.bfloat16
F32 = mybir.dt.float32

N_CORES = 8
T = 2048
D = 2048
HD = 64          # head dim
NH = 32          # total q heads
NKV = 8          # total kv heads
NH_LOC = NH // N_CORES       # 4 q heads per core
QW = NH_LOC * HD             # 256 q dims per core
P = 128
KT = D // P                  # 16 contraction tiles
CH = 512                     # T-chunk
NCH = T // CH                # 4 chunks
NJ = T // P                  # 16 k-blocks
SCALE = 1.0 / 8.0            # 1/sqrt(64)


def build_bass():
    nc = bacc.Bacc(None, num_devices=N_CORES)

    # I/O
    xT_d = nc.dram_tensor("xT", [D, T], BF16, kind="ExternalInput")
    w_d = nc.dram_tensor("w_all", [D, QW + 2 * HD], BF16, kind="ExternalInput")
    woT_d = nc.dram_tensor("woT", [QW, D], BF16, kind="ExternalInput")
    cos2_d = nc.dram_tensor("cos2", [P, T], BF16, kind="ExternalInput")
    sin2_d = nc.dram_tensor("sin2", [P, T], BF16, kind="ExternalInput")
    mask2_d = nc.dram_tensor("dmask2", [P, 4, 2 * CH], BF16, kind="ExternalInput")
    perm_d = nc.dram_tensor("perm", [P, P], BF16, kind="ExternalInput")
    y_d = nc.dram_tensor("y", [QW, T], BF16, kind="ExternalOutput")

    with tile.TileContext(nc, num_cores=N_CORES) as tc:
        _body(nc, tc, xT_d, w_d, woT_d, cos2_d, sin2_d, mask2_d, perm_d, y_d)
    nc.compile()
    return nc


def _body(nc, tc, xT_d, w_d, woT_d, cos2_d, sin2_d, mask2_d, perm_d, y_d):
    import contextlib

    ctx = contextlib.ExitStack()
    with ctx:
        consts = ctx.enter_context(tc.tile_pool(name="consts", bufs=1))
        big = ctx.enter_context(tc.tile_pool(name="big", bufs=1))
        work = ctx.enter_context(tc.tile_pool(name="work", bufs=1))
        rp = ctx.enter_context(tc.tile_pool(name="rp", bufs=2))
        pt_p = ctx.enter_context(tc.tile_pool(name="pt_p", bufs=3))
        nrm = ctx.enter_context(tc.tile_pool(name="nrm", bufs=2))
        pst = ctx.enter_context(tc.tile_pool(name="pst", bufs=2))
        stP = ctx.enter_context(tc.tile_pool(name="stP", bufs=2, space="PSUM"))
        otP = ctx.enter_context(tc.tile_pool(name="otP", bufs=2, space="PSUM"))
        drP = ctx.enter_context(tc.tile_pool(name="drP", bufs=1, space="PSUM"))
        dram = ctx.enter_context(tc.tile_pool(name="dram", bufs=1, space="DRAM"))

        # ---- constant tiles (few big DMAs; the DMA path serializes per
        # dma_start, so count matters more than size)
        w_sb = consts.tile([P, KT, QW + 2 * HD], BF16)
        w_r = w_d.rearrange("(kt p) m -> p kt m", p=P)
        for hf in range(2):
            nc.sync.dma_start(w_sb[:, 8 * hf : 8 * hf + 8, :], w_r[:, 8 * hf : 8 * hf + 8, :])
        xT_sb = big.tile([P, KT, T], BF16, tag="big", name="xT_sb")
        xT_r = xT_d.rearrange("(kt p) t -> p kt t", p=P)
        nc.sync.dma_start(xT_sb[:, :, 0:CH], xT_r[:, :, 0:CH])
        cos2_sb = consts.tile([P, T], BF16)
        nc.sync.dma_start(cos2_sb, cos2_d[:])
        sin2_sb = consts.tile([P, T], BF16)
        nc.sync.dma_start(sin2_sb, sin2_d[:])
        perm_sb = consts.tile([P, P], BF16)
        nc.sync.dma_start(perm_sb, perm_d[:])
        nc.sync.dma_start(xT_sb[:, :, CH : 2 * CH], xT_r[:, :, CH : 2 * CH])
        mask2_sb = consts.tile([P, 4, 2 * CH], BF16)
        nc.sync.dma_start(mask2_sb, mask2_d[:])
        woT2_sb = consts.tile([P, 2, D], BF16)
        woT2_r = woT_d.rearrange("(g p) o -> p g o", p=P)
        nc.sync.dma_start(woT2_sb, woT2_r[:, :, :])
        nc.sync.dma_start(xT_sb[:, :, 2 * CH : 3 * CH], xT_r[:, :, 2 * CH : 3 * CH])
        nc.sync.dma_start(xT_sb[:, :, 3 * CH : 4 * CH], xT_r[:, :, 3 * CH : 4 * CH])

        ones64 = consts.tile([1, HD], BF16)
        nc.vector.memset(ones64, 1.0)

        # v with ones column appended: [tk(P), j, HD+1]
        v_aug = work.tile([P, NJ, HD + 1], BF16)
        nc.vector.memset(v_aug[:, :, HD : HD + 1], 1.0)

        # k^T duplicated on both partition halves so odd heads (whose rope
        # output lives at partitions 64:128) can matmul base-aligned
        kT2 = work.tile([P, T], BF16)
        # O^T repacked to 128 partitions: OT128[64*(h%2)+d, h//2, t]
        # (partition p of pair g is local head-dim 128*g+p, matching woT2)
        OT128 = work.tile([P, 2, T], BF16)

        # chunk-major so each chunk's collective sees contiguous DRAM
        partT_dram = dram.tile([NCH, D, CH], BF16)
        partT_r = partT_dram.rearrange("c (ot p) t -> p c ot t", p=P)
        ysT_dram = dram.tile([NCH, QW, CH], BF16)

        cheap = [nc.scalar, nc.vector]

        def copy_via(idx, dst, src):
            eng = cheap[idx % 2]
            if eng is nc.scalar:
                nc.scalar.copy(dst, src)
            else:
                nc.vector.tensor_copy(dst, src)

        def emit_proj_rope(c):
            """QKV projection + RoPE + v-transpose for chunk c."""
            sl = slice(c * CH, (c + 1) * CH)
            projT = rp.tile([P, 3, CH], BF16, tag="projT")
            for m in (2, 0, 1):
                ps = drP.tile([P, 2 * CH], F32, tag="dr")
                for kt in range(KT):
                    nc.tensor.matmul(
                        ps[:, 0:CH],
                        lhsT=w_sb[:, kt, m * P : (m + 1) * P],
                        rhs=xT_sb[:, kt, sl],
                        start=(kt == 0),
                        stop=(kt == KT - 1),
                    )
                copy_via(m, projT[:, m, :], ps[:, 0:CH])

            # RoPE k (rows 0:64 of projT[:,2]) -> kT_sb[:, sl]
            ksh = drP.tile([P, 2 * CH], F32, tag="dr")
            nc.tensor.matmul(
                ksh[0:HD, 0:CH], lhsT=perm_sb[0:HD, 0:HD],
                rhs=projT[0:HD, 2, :], start=True, stop=True,
            )
            kt2 = rp.tile([P, CH], BF16, tag="t2")
            nc.vector.tensor_mul(kt2[0:HD, :], ksh[0:HD, 0:CH], sin2_sb[0:HD, sl])
            kt1 = rp.tile([P, CH], BF16, tag="t1")
            nc.gpsimd.tensor_mul(kt1[0:HD, :], projT[0:HD, 2, :], cos2_sb[0:HD, sl])
            nc.gpsimd.tensor_add(kT2[0:HD, sl], kt1[0:HD, :], kt2[0:HD, :])
            nc.gpsimd.dma_start(kT2[HD:P, sl], kT2[0:HD, sl])

            # v directly in t-partitioned layout: v[t, d] with x-tiles as the
            # stationary operand (free dim 64 -> 27ns/matmul); avoids the
            # DMA-transpose, which the scheduler serializes with collectives
            for tb in range(4):
                pv = drP.tile([P, 2 * CH], F32, tag="dr")
                for kt in range(KT):
                    nc.tensor.matmul(
                        pv[:, 0:HD],
                        lhsT=xT_sb[:, kt, (4 * c + tb) * P : (4 * c + tb + 1) * P],
                        rhs=w_sb[:, kt, QW + HD : QW + 2 * HD],
                        start=(kt == 0),
                        stop=(kt == KT - 1),
                    )
                nc.vector.tensor_copy(v_aug[:, 4 * c + tb, 0:HD], pv[:, 0:HD])

            # RoPE q pairs: heads (2s, 2s+1) stay at partitions 0:64/64:128
            qpair = []
            for s in range(2):
                qsh = drP.tile([P, 2 * CH], F32, tag="dr")
                nc.tensor.matmul(
                    qsh[:, 0:CH], lhsT=perm_sb, rhs=projT[:, s, :],
                    start=True, stop=True,
                )
                t2 = rp.tile([P, CH], BF16, tag="t2")
                nc.vector.tensor_mul(t2, qsh[:, 0:CH], sin2_sb[:, sl])
                t1 = rp.tile([P, CH], BF16, tag="t1")
                nc.gpsimd.tensor_mul(t1, projT[:, s, :], cos2_sb[:, sl])
                qp = rp.tile([P, CH], BF16, tag="qp%d" % s)
                nc.gpsimd.tensor_add(qp, t1, t2)
                qpair.append(qp)
            return qpair

        def drain_gen(c, final=False):
            """Yield once per schedulable step of chunk c's partial-Wo drain.
            Units of 4 matmuls + 1 copy; partT DMA halves along the way.
            Interleaved into attention (c+1)'s j-loop as PE filler; the final
            drain (no attention left) double-buffers by alternating its PSUM
            between the drP and the now-idle stP pools."""
            sl = slice(c * CH, (c + 1) * CH)
            pstage = pst.tile([P, KT, CH], BF16, tag="pst")
            for ot2 in range(8):
                if final and ot2 % 2 == 1:
                    ps = stP.tile([P, 2, CH], F32, tag="st")
                    ps = ps[:, :, :].rearrange("p a b -> p (a b)")
                else:
                    ps = drP.tile([P, 2 * CH], F32, tag="dr")
                for half in range(2):
                    o_t = 2 * ot2 + half
                    for g in range(2):
                        nc.tensor.matmul(
                            ps[:, half * CH : (half + 1) * CH],
                            lhsT=woT2_sb[:, g, o_t * P : (o_t + 1) * P],
                            rhs=OT128[:, g, sl],
                            start=(g == 0),
                            stop=(g == 1),
                        )
                    yield
                copy_via(ot2, pstage[:, 2 * ot2 : 2 * ot2 + 2, :], ps)
                if ot2 in (3, 7):
                    hb = 8 * (ot2 // 4)
                    nc.sync.dma_start(
                        partT_r[:, c, hb : hb + 8, :], pstage[:, hb : hb + 8, :]
                    )
                yield

        def emit_rs(c):
            """ReduceScatter chunk c + its y output DMAs (emitted at a fixed
            program point so its semaphore wait cannot head-block the Pool
            queue's latency-critical DMAs)."""
            sl = slice(c * CH, (c + 1) * CH)
            nc.gpsimd.collective_compute(
                "ReduceScatter",
                mybir.AluOpType.add,
                replica_groups=[list(range(N_CORES))],
                ins=[partT_dram[c, :, :]],
                outs=[ysT_dram[c, :, :]],
            )
            for r in range(2):
                nc.sync.dma_start(
                    y_d[r * P : (r + 1) * P, sl], ysT_dram[c, r * P : (r + 1) * P, :]
                )

        def emit_attention(c, qpair, filler):
            """Attention for the 4 local heads of q-chunk c, exp grouped over
            2 adjacent k-blocks; pulls from filler (drain of chunk c-1)
            between ST and PV so PE never idles while Exp runs."""
            nj = 4 * c + 4
            stg = nrm.tile([HD + 1, NH_LOC, CH], BF16, tag="stg")
            for h in (0, 2, 1, 3):
                lo = HD * (h % 2)
                qrhs = qpair[h // 2][lo : lo + HD, :]
                ot = otP.tile([P, CH], F32, tag="ot")

                def do_st(j2):
                    st = stP.tile([P, 2, CH], F32, tag="st")
                    for i in range(2):
                        nc.tensor.matmul(
                            st[:, i, :],
                            lhsT=kT2[lo : lo + HD, (2 * j2 + i) * P : (2 * j2 + i + 1) * P],
                            rhs=qrhs,
                            start=True,
                            stop=True,
                        )
                    return st

                def do_rest(st, j2):
                    pt = pt_p.tile([P, 2, CH], BF16, tag="pt")
                    nc.scalar.activation(
                        pt, st, mybir.ActivationFunctionType.Exp, scale=SCALE
                    )
                    for i in range(2):
                        j = 2 * j2 + i
                        if j >= 4 * c:  # diagonal block: zero masked region
                            nc.vector.tensor_mul(
                                pt[:, i, :], pt[:, i, :], mask2_sb[:, j - 4 * c, 0:CH]
                            )
                        nc.tensor.matmul(
                            ot[0 : HD + 1, :],
                            lhsT=v_aug[:, j, :],
                            rhs=pt[:, i, :],
                            start=(j == 0),
                            stop=(j == nj - 1),
                            skip_group_check=True,
                        )

                st_cur = do_st(0)
                for j2 in range(nj // 2):
                    st_next = do_st(j2 + 1) if j2 + 1 < nj // 2 else None
                    next(filler, None)
                    do_rest(st_cur, j2)
                    next(filler, None)
                    st_cur = st_next

                # stage unnormalized O^T + denominator row (bf16)
                if h % 2 == 0:
                    nc.scalar.copy(stg[:, h, :], ot[0 : HD + 1, :])
                else:
                    nc.vector.tensor_copy(stg[:, h, :], ot[0 : HD + 1, :])
            return stg

        def emit_norm(c, stg):
            """Softmax normalization for chunk c (batched over 4 heads)."""
            sl = slice(c * CH, (c + 1) * CH)
            denT = nrm.tile([1, NH_LOC, CH], BF16, tag="den")
            nc.gpsimd.dma_start(denT[0:1, :, :], stg[HD : HD + 1, :, :])
            rcpT = nrm.tile([1, NH_LOC, CH], BF16, tag="rcp")
            with nc.allow_low_precision("softmax denom in bf16 is fine"):
                nc.vector.reciprocal(rcpT, denT)
            for h in range(NH_LOC):
                bc = otP.tile([P, CH], F32, tag="ot")
                nc.tensor.matmul(
                    bc[0:HD, :], lhsT=ones64, rhs=rcpT[0:1, h, :],
                    start=True, stop=True,
                )
                if h % 2 == 0:
                    nc.vector.tensor_mul(
                        OT128[0:HD, h // 2, sl], stg[0:HD, h, :], bc[0:HD, :]
                    )
                else:
                    oddt = nrm.tile([HD, CH], BF16, tag="oddt")
                    nc.vector.tensor_mul(oddt, stg[0:HD, h, :], bc[0:HD, :])
                    nc.gpsimd.dma_start(OT128[HD:P, h // 2, sl], oddt)

        # ---- software-pipelined main loop:
        #   attention c (draining c-1 on PE bubbles) -> proj/rope c+1
        #   (hides norm c's DMA chain) -> norm c
        qpair = emit_proj_rope(0)
        filler = iter(())
        for c in range(NCH):
            stg = emit_attention(c, qpair, filler)
            for _ in filler:
                pass
            if c + 1 < NCH:
                qpair = emit_proj_rope(c + 1)
            if c >= 1:
                emit_rs(c - 1)
            emit_norm(c, stg)
            filler = drain_gen(c, final=(c == NCH - 1))
        # final drain (chunk 3) runs dense
        for _ in filler:
            pass
        emit_rs(NCH - 1)

def _prep_shards(x, Wq, lora_A, lora_B, Wk, Wv, Wo):
    bf16 = ml_dtypes.bfloat16
    xT = np.ascontiguousarray(x[0].T).astype(bf16)

    theta = 1.0 / (10000.0 ** (np.arange(0, HD, 2, dtype=np.float32) / HD))
    pos = np.arange(T, dtype=np.float32)
    ang = pos[:, None] * theta[None, :]
    ang = np.concatenate([ang, ang], axis=-1)          # [T, HD]
    cosT = np.cos(ang).T                               # [HD, T]
    sinT = np.sin(ang).T                               # unsigned; sign in perm
    cos2 = np.ascontiguousarray(np.concatenate([cosT, cosT], 0)).astype(bf16)
    sin2 = np.ascontiguousarray(np.concatenate([sinT, sinT], 0)).astype(bf16)

    # signed rotate-half permutation (per 64-row head block):
    # out[p] = sign(p) * src[rot(p)], rot = +-32 within the block
    perm = np.zeros((P, P), dtype=np.float32)
    for p in range(P):
        blk, q = (p // HD) * HD, p % HD
        rot = blk + (q + 32) % HD
        perm[rot, p] = -1.0 if q < 32 else 1.0
    perm = perm.astype(bf16)

    p_idx = np.arange(P)[:, None, None]
    m_idx = np.arange(4)[None, :, None]
    f_idx = np.arange(CH)[None, None, :]
    dmask = (p_idx + P * m_idx <= f_idx).astype(bf16)  # [128, 4, 512]
    dmask2 = np.concatenate([dmask, dmask], axis=2)    # [128, 4, 1024]

    Wq_eff = Wq + lora_B.astype(np.float64) @ lora_A.astype(np.float64)
    Wq_eff = Wq_eff.astype(np.float32)

    in_maps = []
    for i in range(N_CORES):
        wq_i = Wq_eff[QW * i : QW * (i + 1), :]        # [256, D]
        wk_i = Wk[HD * i : HD * (i + 1), :]            # [64, D]
        wv_i = Wv[HD * i : HD * (i + 1), :]
        w_all = np.ascontiguousarray(
            np.concatenate([wq_i, wk_i, wv_i], 0).T
        ).astype(bf16)                                 # [D, 384]
        # Wo^T rows for this core's head-dims: [256, D]
        woT = np.ascontiguousarray(Wo[:, QW * i : QW * (i + 1)].T).astype(bf16)
        in_maps.append({
            "xT": xT,
            "w_all": w_all,
            "woT": woT,
            "cos2": cos2,
            "sin2": sin2,
            "dmask2": dmask2,
            "perm": perm,
        })
    return in_maps


def run(inputs, trace=False, **kw):
    nc = build_bass()
    in_maps = _prep_shards(**inputs)
    res = run_bass_kernel_spmd(
        nc, in_maps, core_ids=list(range(N_CORES)), trace=trace, **kw
    )
    # core i returns y^T rows [256*i, 256*(i+1)) = y columns
    y = np.concatenate(
        [np.asarray(res.results[i]["y"]).astype(np.float32).T for i in range(N_CORES)],
        axis=1,
    )
    return y[None], res


def kernel(**inputs):
    y, _ = run(inputs)
    return y


# revision 39
# speedup vs baseline: 1.8379x; 1.0461x over previous
"""GQA attention with LoRA-Q, tensor-parallel over 8 TRN2 cores.

Sharding (per core i of 8):
  - Q heads 4i..4i+3 (256 q-dims) and KV head i (GQA: repeat_interleave maps
    q heads [4i,4i+4) exactly onto kv head i).
  - Wq (with LoRA folded: Wq_eff = Wq + lora_B @ lora_A), Wk, Wv row-sharded;
    Wo row-parallel on its input (head) dim: each core computes the full-width
    partial y^T from its own 256 head-dims; four per-T-chunk ReduceScatter(add)
    ops (overlapped with attention of later chunks) leave each core its
    256-feature slice of y^T (transposed + concatenated on the host).

Structure is a single fused c-outer pipeline over the four 512-token chunks:
  proj c -> rope c -> attention (head pairs, shared kv) c -> norm c ->
  partial-Wo drain c -> ReduceScatter #c
so the collective and DMA traffic hide under attention of later chunks.

All matmuls in bf16 with fp32 PSUM accumulation; softmax without max
subtraction (scores are bounded: |S/8| <= ~7), denominator fused into the
PV matmul via an appended ones-column on V. RoPE's rotate-half is a signed
permutation matmul on PE (no partition-shuffle DMAs).
"""

import numpy as np
import ml_dtypes

import concourse.bass as bass
import concourse.mybir as mybir
import concourse.tile as tile
from concourse import bacc
from concourse.bass_utils import run_bass_kernel_spmd

BF16 = mybir.dt.bfloat16
F32 = mybir.dt.float32
FP8 = mybir.dt.float8e4

N_CORES = 8
T = 2048
D = 2048
HD = 64          # head dim
NH = 32          # total q heads
NKV = 8          # total kv heads
NH_LOC = NH // N_CORES       # 4 q heads per core
QW = NH_LOC * HD             # 256 q dims per core
P = 128
KT = D // P                  # 16 contraction tiles
CH = 512                     # T-chunk
NCH = T // CH                # 4 chunks
NJ = T // P                  # 16 k-blocks
SCALE = 1.0 / 8.0            # 1/sqrt(64)


def build_bass():
    nc = bacc.Bacc(None, num_devices=N_CORES)

    # I/O
    xT_d = nc.dram_tensor("xT", [D, T], BF16, kind="ExternalInput")
    w_d = nc.dram_tensor("w_all", [D, QW + 2 * HD], BF16, kind="ExternalInput")
    woT_d = nc.dram_tensor("woT", [QW, D], BF16, kind="ExternalInput")
    cos2_d = nc.dram_tensor("cos2", [P, T], BF16, kind="ExternalInput")
    sin2_d = nc.dram_tensor("sin2", [P, T], BF16, kind="ExternalInput")
    mask2_d = nc.dram_tensor("dmask2", [P, 4, 2 * CH], BF16, kind="ExternalInput")
    perm_d = nc.dram_tensor("perm", [P, P], BF16, kind="ExternalInput")
    y_d = nc.dram_tensor("y", [QW, T], BF16, kind="ExternalOutput")

    with tile.TileContext(nc, num_cores=N_CORES) as tc:
        _body(nc, tc, xT_d, w_d, woT_d, cos2_d, sin2_d, mask2_d, perm_d, y_d)
    nc.compile()
    return nc


def _body(nc, tc, xT_d, w_d, woT_d, cos2_d, sin2_d, mask2_d, perm_d, y_d):
    import contextlib

    ctx = contextlib.ExitStack()
    with ctx:
        consts = ctx.enter_context(tc.tile_pool(name="consts", bufs=1))
        big = ctx.enter_context(tc.tile_pool(name="big", bufs=1))
        work = ctx.enter_context(tc.tile_pool(name="work", bufs=1))
        rp = ctx.enter_context(tc.tile_pool(name="rp", bufs=2))
        pt_p = ctx.enter_context(tc.tile_pool(name="pt_p", bufs=3))
        nrm = ctx.enter_context(tc.tile_pool(name="nrm", bufs=2))
        pst = ctx.enter_context(tc.tile_pool(name="pst", bufs=2))
        stP = ctx.enter_context(tc.tile_pool(name="stP", bufs=2, space="PSUM"))
        otP = ctx.enter_context(tc.tile_pool(name="otP", bufs=2, space="PSUM"))
        drP = ctx.enter_context(tc.tile_pool(name="drP", bufs=1, space="PSUM"))
        dram = ctx.enter_context(tc.tile_pool(name="dram", bufs=1, space="DRAM"))

        # ---- constant tiles (few big DMAs; the DMA path serializes per
        # dma_start, so count matters more than size)
        w_sb = consts.tile([P, KT, QW + 2 * HD], BF16)
        w_r = w_d.rearrange("(kt p) m -> p kt m", p=P)
        for hf in range(2):
            nc.sync.dma_start(w_sb[:, 8 * hf : 8 * hf + 8, :], w_r[:, 8 * hf : 8 * hf + 8, :])
        xT_sb = big.tile([P, KT, T], BF16, tag="big", name="xT_sb")
        xT_r = xT_d.rearrange("(kt p) t -> p kt t", p=P)
        nc.sync.dma_start(xT_sb[:, :, 0:CH], xT_r[:, :, 0:CH])
        cos2_sb = consts.tile([P, T], BF16)
        nc.sync.dma_start(cos2_sb, cos2_d[:])
        sin2_sb = consts.tile([P, T], BF16)
        nc.sync.dma_start(sin2_sb, sin2_d[:])
        perm_sb = consts.tile([P, P], BF16)
        nc.sync.dma_start(perm_sb, perm_d[:])
        nc.sync.dma_start(xT_sb[:, :, CH : 2 * CH], xT_r[:, :, CH : 2 * CH])
        tri_sb = consts.tile([P, P], BF16)
        nc.sync.dma_start(tri_sb, mask2_d[0:P, 0, 0:P])
        woT2_sb = consts.tile([P, 2, D], BF16)
        woT2_r = woT_d.rearrange("(g p) o -> p g o", p=P)
        nc.sync.dma_start(woT2_sb, woT2_r[:, :, :])
        nc.sync.dma_start(xT_sb[:, :, 2 * CH : 3 * CH], xT_r[:, :, 2 * CH : 3 * CH])
        nc.sync.dma_start(xT_sb[:, :, 3 * CH : 4 * CH], xT_r[:, :, 3 * CH : 4 * CH])

        ones64 = consts.tile([1, HD], BF16)
        nc.vector.memset(ones64, 1.0)

        # v with ones column appended: [tk(P), j, HD+1]
        v_aug = work.tile([P, NJ, HD + 1], BF16)
        nc.vector.memset(v_aug[:, :, HD : HD + 1], 1.0)

        # k^T duplicated on both partition halves so odd heads (whose rope
        # output lives at partitions 64:128) can matmul base-aligned
        kT2 = work.tile([P, T], BF16)
        # O^T repacked to 128 partitions: OT128[64*(h%2)+d, h//2, t]
        # (partition p of pair g is local head-dim 128*g+p, matching woT2)
        OT128 = work.tile([P, 2, T], BF16)

        # chunk-major so each chunk's collective sees contiguous DRAM
        partT_dram = dram.tile([NCH, D, CH], BF16)
        partT_r = partT_dram.rearrange("c (ot p) t -> p c ot t", p=P)
        ysT_dram = dram.tile([NCH, QW, CH], BF16)

        def copy_via(idx, dst, src, act_every=2):
            if idx % act_every == 0:
                nc.scalar.copy(dst, src)
            else:
                nc.vector.tensor_copy(dst, src)

        def emit_proj_rope(c):
            """QKV projection + RoPE + v-transpose for chunk c."""
            sl = slice(c * CH, (c + 1) * CH)
            projT = rp.tile([P, 3, CH], BF16, tag="projT")
            for m in (2, 0, 1):
                ps = drP.tile([P, 2 * CH], F32, tag="dr")
                for kt in range(KT):
                    nc.tensor.matmul(
                        ps[:, 0:CH],
                        lhsT=w_sb[:, kt, m * P : (m + 1) * P],
                        rhs=xT_sb[:, kt, sl],
                        start=(kt == 0),
                        stop=(kt == KT - 1),
                    )
                copy_via(m, projT[:, m, :], ps[:, 0:CH])

            # RoPE k (rows 0:64 of projT[:,2]) -> kT_sb[:, sl]
            ksh = drP.tile([P, 2 * CH], F32, tag="dr")
            nc.tensor.matmul(
                ksh[0:HD, 0:CH], lhsT=perm_sb[0:HD, 0:HD],
                rhs=projT[0:HD, 2, :], start=True, stop=True,
            )
            kt2 = rp.tile([P, CH], BF16, tag="t2")
            nc.vector.tensor_mul(kt2[0:HD, :], ksh[0:HD, 0:CH], sin2_sb[0:HD, sl])
            kt1 = rp.tile([P, CH], BF16, tag="t1")
            nc.gpsimd.tensor_mul(kt1[0:HD, :], projT[0:HD, 2, :], cos2_sb[0:HD, sl])
            nc.gpsimd.tensor_add(kT2[0:HD, sl], kt1[0:HD, :], kt2[0:HD, :])
            nc.gpsimd.dma_start(kT2[HD:P, sl], kT2[0:HD, sl])

            # v directly in t-partitioned layout: v[t, d] with x-tiles as the
            # stationary operand (free dim 64 -> 27ns/matmul); avoids the
            # DMA-transpose, which the scheduler serializes with collectives
            for tb in range(4):
                pv = drP.tile([P, 2 * CH], F32, tag="dr")
                for kt in range(KT):
                    nc.tensor.matmul(
                        pv[:, 0:HD],
                        lhsT=xT_sb[:, kt, (4 * c + tb) * P : (4 * c + tb + 1) * P],
                        rhs=w_sb[:, kt, QW + HD : QW + 2 * HD],
                        start=(kt == 0),
                        stop=(kt == KT - 1),
                    )
                nc.vector.tensor_copy(v_aug[:, 4 * c + tb, 0:HD], pv[:, 0:HD])

            # RoPE q pairs: heads (2s, 2s+1) stay at partitions 0:64/64:128
            qpair = []
            for s in range(2):
                qsh = drP.tile([P, 2 * CH], F32, tag="dr")
                nc.tensor.matmul(
                    qsh[:, 0:CH], lhsT=perm_sb, rhs=projT[:, s, :],
                    start=True, stop=True,
                )
                t2 = rp.tile([P, CH], BF16, tag="t2")
                nc.vector.tensor_mul(t2, qsh[:, 0:CH], sin2_sb[:, sl])
                t1 = rp.tile([P, CH], BF16, tag="t1")
                nc.gpsimd.tensor_mul(t1, projT[:, s, :], cos2_sb[:, sl])
                qp = rp.tile([P, CH], BF16, tag="qp%d" % s)
                nc.gpsimd.tensor_add(qp, t1, t2)
                qpair.append(qp)
            return qpair

        def drain_gen(c, final=False):
            """Yield once per schedulable step of chunk c's partial-Wo drain.
            Units of 4 matmuls + 1 copy; partT DMA halves along the way.
            Interleaved into attention (c+1)'s j-loop as PE filler; the final
            drain (no attention left) double-buffers by alternating its PSUM
            between the drP and the now-idle stP pools."""
            sl = slice(c * CH, (c + 1) * CH)
            pstage = pst.tile([P, KT, CH], BF16, tag="pst")
            for ot2 in range(8):
                if final and ot2 % 2 == 1:
                    ps = stP.tile([P, 2, CH], F32, tag="st")
                    ps = ps[:, :, :].rearrange("p a b -> p (a b)")
                else:
                    ps = drP.tile([P, 2 * CH], F32, tag="dr")
                for half in range(2):
                    o_t = 2 * ot2 + half
                    for g in range(2):
                        nc.tensor.matmul(
                            ps[:, half * CH : (half + 1) * CH],
                            lhsT=woT2_sb[:, g, o_t * P : (o_t + 1) * P],
                            rhs=OT128[:, g, sl],
                            start=(g == 0),
                            stop=(g == 1),
                        )
                    yield
                copy_via(ot2 + 1, pstage[:, 2 * ot2 : 2 * ot2 + 2, :], ps, act_every=4)
                if ot2 in (3, 7):
                    hb = 8 * (ot2 // 4)
                    nc.sync.dma_start(
                        partT_r[:, c, hb : hb + 8, :], pstage[:, hb : hb + 8, :]
                    )
                yield

        def emit_rs(c):
            """ReduceScatter chunk c + its y output DMAs (emitted at a fixed
            program point so its semaphore wait cannot head-block the Pool
            queue's latency-critical DMAs)."""
            sl = slice(c * CH, (c + 1) * CH)
            nc.gpsimd.collective_compute(
                "ReduceScatter",
                mybir.AluOpType.add,
                replica_groups=[list(range(N_CORES))],
                ins=[partT_dram[c, :, :]],
                outs=[ysT_dram[c, :, :]],
            )
            for r in range(2):
                nc.sync.dma_start(
                    y_d[r * P : (r + 1) * P, sl], ysT_dram[c, r * P : (r + 1) * P, :]
                )

        def emit_attention(c, qpair, filler):
            """Attention for the 4 local heads of q-chunk c, exp grouped over
            2 adjacent k-blocks; pulls from filler (drain of chunk c-1)
            between ST and PV so PE never idles while Exp runs."""
            nj = 4 * c + 4
            stg = nrm.tile([HD + 1, NH_LOC, CH], BF16, tag="stg")
            for h in (0, 2, 1, 3):
                lo = HD * (h % 2)
                qrhs = qpair[h // 2][lo : lo + HD, :]
                ot = otP.tile([P, CH], F32, tag="ot")

                def do_st(j2):
                    st = stP.tile([P, 2, CH], F32, tag="st")
                    for i in range(2):
                        nc.tensor.matmul(
                            st[:, i, :],
                            lhsT=kT2[lo : lo + HD, (2 * j2 + i) * P : (2 * j2 + i + 1) * P],
                            rhs=qrhs,
                            start=True,
                            stop=True,
                        )
                    return st

                def do_rest(st, j2):
                    pt = pt_p.tile([P, 2, CH], BF16, tag="pt")
                    nc.scalar.activation(
                        pt, st, mybir.ActivationFunctionType.Exp, scale=SCALE
                    )
                    if 2 * j2 >= 4 * c:
                        # diagonal group: per-block column trim. Block at
                        # diag offset d contributes nothing to cols < 128d;
                        # only the 128-wide triangle at [128d, 128d+128)
                        # needs masking; cols >= 128(d+1) are fully valid.
                        for i in range(2):
                            j = 2 * j2 + i
                            d = j - 4 * c
                            nc.vector.tensor_mul(
                                pt[:, i, P * d : P * (d + 1)],
                                pt[:, i, P * d : P * (d + 1)],
                                tri_sb,
                            )
                            nc.tensor.matmul(
                                ot[0 : HD + 1, P * d : CH],
                                lhsT=v_aug[:, j, :],
                                rhs=pt[:, i, P * d : CH],
                                start=(j == 0),
                                stop=(j == nj - 1),
                                skip_group_check=True,
                            )
                    else:
                        for i in range(2):
                            j = 2 * j2 + i
                            nc.tensor.matmul(
                                ot[0 : HD + 1, :],
                                lhsT=v_aug[:, j, :],
                                rhs=pt[:, i, :],
                                start=(j == 0),
                                stop=False,
                                skip_group_check=True,
                            )

                st_cur = do_st(0)
                for j2 in range(nj // 2):
                    st_next = do_st(j2 + 1) if j2 + 1 < nj // 2 else None
                    next(filler, None)
                    do_rest(st_cur, j2)
                    next(filler, None)
                    st_cur = st_next

                # stage unnormalized O^T + denominator row (bf16)
                copy_via(h, stg[:, h, :], ot[0 : HD + 1, :])
            return stg

        def emit_norm(c, stg):
            """Softmax normalization for chunk c (batched over 4 heads)."""
            sl = slice(c * CH, (c + 1) * CH)
            denT = nrm.tile([1, NH_LOC, CH], BF16, tag="den")
            nc.gpsimd.dma_start(denT[0:1, :, :], stg[HD : HD + 1, :, :])
            rcpT = nrm.tile([1, NH_LOC, CH], BF16, tag="rcp")
            with nc.allow_low_precision("softmax denom in bf16 is fine"):
                nc.vector.reciprocal(rcpT, denT)
            for h in range(NH_LOC):
                bc = otP.tile([P, CH], F32, tag="ot")
                nc.tensor.matmul(
                    bc[0:HD, :], lhsT=ones64, rhs=rcpT[0:1, h, :],
                    start=True, stop=True,
                )
                if h % 2 == 0:
                    nc.vector.tensor_mul(
                        OT128[0:HD, h // 2, sl], stg[0:HD, h, :], bc[0:HD, :]
                    )
                else:
                    oddt = nrm.tile([HD, CH], BF16, tag="oddt")
                    nc.vector.tensor_mul(oddt, stg[0:HD, h, :], bc[0:HD, :])
                    nc.gpsimd.dma_start(OT128[HD:P, h // 2, sl], oddt)

        # ---- software-pipelined main loop:
        #   attention c (draining c-1 on PE bubbles) -> proj/rope c+1
        #   (hides norm c's DMA chain) -> norm c
        qpair = emit_proj_rope(0)
        filler = iter(())
        for c in range(NCH):
            stg = emit_attention(c, qpair, filler)
            for _ in filler:
                pass
            if c + 1 < NCH:
                qpair = emit_proj_rope(c + 1)
            if c >= 1:
                emit_rs(c - 1)
            emit_norm(c, stg)
            filler = drain_gen(c, final=(c == NCH - 1))
        # final drain (chunk 3) runs dense
        for _ in filler:
            pass
        emit_rs(NCH - 1)

def _prep_shards(x, Wq, lora_A, lora_B, Wk, Wv, Wo):
    bf16 = ml_dtypes.bfloat16
    xT = np.ascontiguousarray(x[0].T).astype(bf16)

    theta = 1.0 / (10000.0 ** (np.arange(0, HD, 2, dtype=np.float32) / HD))
    pos = np.arange(T, dtype=np.float32)
    ang = pos[:, None] * theta[None, :]
    ang = np.concatenate([ang, ang], axis=-1)          # [T, HD]
    cosT = np.cos(ang).T                               # [HD, T]
    sinT = np.sin(ang).T                               # unsigned; sign in perm
    cos2 = np.ascontiguousarray(np.concatenate([cosT, cosT], 0)).astype(bf16)
    sin2 = np.ascontiguousarray(np.concatenate([sinT, sinT], 0)).astype(bf16)

    # signed rotate-half permutation (per 64-row head block):
    # out[p] = sign(p) * src[rot(p)], rot = +-32 within the block
    perm = np.zeros((P, P), dtype=np.float32)
    for p in range(P):
        blk, q = (p // HD) * HD, p % HD
        rot = blk + (q + 32) % HD
        perm[rot, p] = -1.0 if q < 32 else 1.0
    perm = perm.astype(bf16)

    p_idx = np.arange(P)[:, None, None]
    m_idx = np.arange(4)[None, :, None]
    f_idx = np.arange(CH)[None, None, :]
    dmask = (p_idx + P * m_idx <= f_idx).astype(bf16)  # [128, 4, 512]
    dmask2 = np.concatenate([dmask, dmask], axis=2)

    Wq_eff = Wq + lora_B.astype(np.float64) @ lora_A.astype(np.float64)
    Wq_eff = Wq_eff.astype(np.float32)

    in_maps = []
    for i in range(N_CORES):
        wq_i = Wq_eff[QW * i : QW * (i + 1), :]        # [256, D]
        wk_i = Wk[HD * i : HD * (i + 1), :]            # [64, D]
        wv_i = Wv[HD * i : HD * (i + 1), :]
        w_all = np.ascontiguousarray(
            np.concatenate([wq_i, wk_i, wv_i], 0).T
        ).astype(bf16)                                 # [D, 384]
        # Wo^T rows for this core's head-dims: [256, D]
        woT = np.ascontiguousarray(Wo[:, QW * i : QW * (i + 1)].T).astype(bf16)
        in_maps.append({
            "xT": xT,
            "w_all": w_all,
            "woT": woT,
            "cos2": cos2,
            "sin2": sin2,
            "dmask2": dmask2,
            "perm": perm,
        })
    return in_maps


def run(inputs, trace=False, **kw):
    nc = build_bass()
    in_maps = _prep_shards(**inputs)
    res = run_bass_kernel_spmd(
        nc, in_maps, core_ids=list(range(N_CORES)), trace=trace, **kw
    )
    # core i returns y^T rows [256*i, 256*(i+1)) = y columns
    y = np.concatenate(
        [np.asarray(res.results[i]["y"]).astype(np.float32).T for i in range(N_CORES)],
        axis=1,
    )
    return y[None], res


def kernel(**inputs):
    y, _ = run(inputs)
    return y


# revision 56
# speedup vs baseline: 2.0215x; 1.0999x over previous
"""GQA attention with LoRA-Q, tensor-parallel over 8 TRN2 cores.

Sharding (per core i of 8):
  - Q heads 4i..4i+3 (256 q-dims) and KV head i (GQA: repeat_interleave maps
    q heads [4i,4i+4) exactly onto kv head i).
  - Wq (with LoRA folded: Wq_eff = Wq + lora_B @ lora_A), Wk, Wv row-sharded;
    Wo row-parallel on its input (head) dim: each core computes the full-width
    partial y^T from its own 256 head-dims; four per-T-chunk ReduceScatter(add)
    ops (overlapped with attention of later chunks) leave each core its
    256-feature slice of y^T (transposed + concatenated on the host).

Structure is a single fused c-outer pipeline over the four 512-token chunks:
  proj c -> rope c -> attention (head pairs, shared kv) c -> norm c ->
  partial-Wo drain c -> ReduceScatter #c
so the collective and DMA traffic hide under attention of later chunks.

All matmuls in bf16 with fp32 PSUM accumulation; softmax without max
subtraction (scores are bounded: |S/8| <= ~7), denominator fused into the
PV matmul via an appended ones-column on V. RoPE's rotate-half is a signed
permutation matmul on PE (no partition-shuffle DMAs).
"""

import numpy as np
import ml_dtypes

import concourse.bass as bass
import concourse.mybir as mybir
import concourse.tile as tile
from concourse import bacc
from concourse.bass_utils import run_bass_kernel_spmd

BF16 = mybir.dt.bfloat16
F32 = mybir.dt.float32
FP8 = mybir.dt.float8e4

N_CORES = 8
T = 2048
D = 2048
HD = 64          # head dim
NH = 32          # total q heads
NKV = 8          # total kv heads
NH_LOC = NH // N_CORES       # 4 q heads per core
QW = NH_LOC * HD             # 256 q dims per core
P = 128
KT = D // P                  # 16 contraction tiles
CH = 512                     # T-chunk
NCH = T // CH                # 4 chunks
NJ = T // P                  # 16 k-blocks
SCALE = 1.0 / 8.0            # 1/sqrt(64)


def build_bass():
    nc = bacc.Bacc(None, num_devices=N_CORES)

    # I/O
    xT_d = nc.dram_tensor("xT", [D, T], BF16, kind="ExternalInput")
    w_d = nc.dram_tensor("w_all", [D, QW + 2 * HD], BF16, kind="ExternalInput")
    woT_d = nc.dram_tensor("woT", [QW, D], BF16, kind="ExternalInput")
    cos2_d = nc.dram_tensor("cos2", [P, T], BF16, kind="ExternalInput")
    sin2_d = nc.dram_tensor("sin2", [P, T], BF16, kind="ExternalInput")
    mask2_d = nc.dram_tensor("dmask2", [P, 4, 2 * CH], BF16, kind="ExternalInput")
    perm_d = nc.dram_tensor("perm", [P, P], BF16, kind="ExternalInput")
    y_d = nc.dram_tensor("y", [NCH, QW, CH], BF16, kind="ExternalOutput")

    with tile.TileContext(nc, num_cores=N_CORES) as tc:
        _body(nc, tc, xT_d, w_d, woT_d, cos2_d, sin2_d, mask2_d, perm_d, y_d)
    nc.compile()
    return nc


def _body(nc, tc, xT_d, w_d, woT_d, cos2_d, sin2_d, mask2_d, perm_d, y_d):
    import contextlib
    import itertools

    ctx = contextlib.ExitStack()
    with ctx:
        consts = ctx.enter_context(tc.tile_pool(name="consts", bufs=1))
        big = ctx.enter_context(tc.tile_pool(name="big", bufs=1))
        work = ctx.enter_context(tc.tile_pool(name="work", bufs=1))
        rp = ctx.enter_context(tc.tile_pool(name="rp", bufs=2))
        pt_p = ctx.enter_context(tc.tile_pool(name="pt_p", bufs=3))
        nrm = ctx.enter_context(tc.tile_pool(name="nrm", bufs=2))
        pst = ctx.enter_context(tc.tile_pool(name="pst", bufs=2))
        stP = ctx.enter_context(tc.tile_pool(name="stP", bufs=2, space="PSUM"))
        otP = ctx.enter_context(tc.tile_pool(name="otP", bufs=2, space="PSUM"))
        drP = ctx.enter_context(tc.tile_pool(name="drP", bufs=1, space="PSUM"))
        dram = ctx.enter_context(tc.tile_pool(name="dram", bufs=1, space="DRAM"))

        # ---- constant tiles (few big DMAs; the DMA path serializes per
        # dma_start, so count matters more than size)
        w_sb = consts.tile([P, KT, QW + 2 * HD], BF16)
        w_r = w_d.rearrange("(kt p) m -> p kt m", p=P)
        for hf in range(2):
            nc.sync.dma_start(w_sb[:, 8 * hf : 8 * hf + 8, :], w_r[:, 8 * hf : 8 * hf + 8, :])
        xT_sb = big.tile([P, KT, T], BF16, tag="big", name="xT_sb")
        xT_r = xT_d.rearrange("(kt p) t -> p kt t", p=P)
        nc.sync.dma_start(xT_sb[:, :, 0:CH], xT_r[:, :, 0:CH])
        cos2_sb = consts.tile([P, T], BF16)
        nc.sync.dma_start(cos2_sb, cos2_d[:])
        sin2_sb = consts.tile([P, T], BF16)
        nc.sync.dma_start(sin2_sb, sin2_d[:])
        perm_sb = consts.tile([P, P], BF16)
        nc.sync.dma_start(perm_sb, perm_d[:])
        nc.sync.dma_start(xT_sb[:, :, CH : 2 * CH], xT_r[:, :, CH : 2 * CH])
        tri_sb = consts.tile([P, P], BF16)
        nc.sync.dma_start(tri_sb, mask2_d[0:P, 0, 0:P])
        woT2_sb = consts.tile([P, 2, D], BF16)
        woT2_r = woT_d.rearrange("(g p) o -> p g o", p=P)
        nc.sync.dma_start(woT2_sb, woT2_r[:, :, :])
        nc.sync.dma_start(xT_sb[:, :, 2 * CH : 3 * CH], xT_r[:, :, 2 * CH : 3 * CH])
        nc.sync.dma_start(xT_sb[:, :, 3 * CH : 4 * CH], xT_r[:, :, 3 * CH : 4 * CH])

        ones64 = consts.tile([1, HD], BF16)
        nc.vector.memset(ones64, 1.0)

        # v with ones column appended: [tk(P), j, HD+1]
        v_aug = work.tile([P, NJ, HD + 1], BF16)
        nc.vector.memset(v_aug[:, :, HD : HD + 1], 1.0)

        # k^T duplicated on both partition halves so odd heads (whose rope
        # output lives at partitions 64:128) can matmul base-aligned
        kT2 = work.tile([P, T], BF16)
        # rope'd q pairs for all chunks: qAll[64*(h%2)+d, h//2, t]
        qAll = work.tile([P, 2, T], BF16)
        # O^T repacked to 128 partitions: OT128[64*(h%2)+d, h//2, t]
        # (partition p of pair g is local head-dim 128*g+p, matching woT2)
        OT128 = work.tile([P, 2, T], BF16)

        # chunk-major so each chunk's collective sees contiguous DRAM
        partT_dram = dram.tile([NCH, D, CH], BF16)
        partT_r = partT_dram.rearrange("c (ot p) t -> p c ot t", p=P)
        ysT_dram = dram.tile([NCH, QW, CH], BF16)

        def copy_via(idx, dst, src, act_every=2):
            if idx % act_every == 0:
                nc.scalar.copy(dst, src)
            else:
                nc.vector.tensor_copy(dst, src)

        def emit_proj_rope(c):
            """QKV projection + RoPE + t-partitioned V for chunk c, as a
            generator so chunks >= 1 can ride attention's PE bubbles."""
            sl = slice(c * CH, (c + 1) * CH)
            projT = rp.tile([P, 3, CH], BF16, tag="projT")
            for m in (2, 0, 1):
                ps = drP.tile([P, 2 * CH], F32, tag="dr")
                for kt in range(KT):
                    nc.tensor.matmul(
                        ps[:, 0:CH],
                        lhsT=w_sb[:, kt, m * P : (m + 1) * P],
                        rhs=xT_sb[:, kt, sl],
                        start=(kt == 0),
                        stop=(kt == KT - 1),
                    )
                    if kt % 8 == 7:
                        yield
                nc.vector.tensor_copy(projT[:, m, :], ps[:, 0:CH])
                yield
                if m == 2:
                    # RoPE k -> kT2[0:64, sl] + dup to [64:128]
                    ksh = drP.tile([P, 2 * CH], F32, tag="dr")
                    nc.tensor.matmul(
                        ksh[0:HD, 0:CH], lhsT=perm_sb[0:HD, 0:HD],
                        rhs=projT[0:HD, 2, :], start=True, stop=True,
                    )
                    kt2 = rp.tile([P, CH], BF16, tag="t2")
                    nc.vector.tensor_mul(kt2[0:HD, :], ksh[0:HD, 0:CH], sin2_sb[0:HD, sl])
                    veng = nc.vector if c == 0 else nc.gpsimd
                    kt1 = rp.tile([P, CH], BF16, tag="t1")
                    veng.tensor_mul(kt1[0:HD, :], projT[0:HD, 2, :], cos2_sb[0:HD, sl])
                    veng.tensor_add(kT2[0:HD, sl], kt1[0:HD, :], kt2[0:HD, :])
                    nc.gpsimd.dma_start(kT2[HD:P, sl], kT2[0:HD, sl])
                    yield
                    # v in t-partitioned layout (free dim 64 -> 27ns/matmul)
                    for tb in range(4):
                        pv = drP.tile([P, 2 * CH], F32, tag="dr")
                        for kt in range(KT):
                            nc.tensor.matmul(
                                pv[:, 0:HD],
                                lhsT=xT_sb[:, kt, (4 * c + tb) * P : (4 * c + tb + 1) * P],
                                rhs=w_sb[:, kt, QW + HD : QW + 2 * HD],
                                start=(kt == 0),
                                stop=(kt == KT - 1),
                            )
                        nc.vector.tensor_copy(v_aug[:, 4 * c + tb, 0:HD], pv[:, 0:HD])
                        yield
                else:
                    s = m
                    qsh = drP.tile([P, 2 * CH], F32, tag="dr")
                    nc.tensor.matmul(
                        qsh[:, 0:CH], lhsT=perm_sb, rhs=projT[:, s, :],
                        start=True, stop=True,
                    )
                    yield
                    t2 = rp.tile([P, CH], BF16, tag="t2")
                    nc.vector.tensor_mul(t2, qsh[:, 0:CH], sin2_sb[:, sl])
                    veng = nc.vector if c == 0 else nc.gpsimd
                    t1 = rp.tile([P, CH], BF16, tag="t1")
                    veng.tensor_mul(t1, projT[:, s, :], cos2_sb[:, sl])
                    veng.tensor_add(qAll[:, s, sl], t1, t2)
                    yield

        def drain_gen(c, final=False):
            """Chunk c's partial-Wo drain + partT DMAs + ReduceScatter.
            Interleaved into attention (c+1)'s j-loop as PE filler; the final
            drain double-buffers by alternating PSUM between drP and the
            idle stP."""
            sl = slice(c * CH, (c + 1) * CH)
            pstage = pst.tile([P, KT, CH], BF16, tag="pst")
            for ot2 in range(8):
                if final and ot2 % 2 == 1:
                    ps = stP.tile([P, 2, CH], F32, tag="st")
                    ps = ps[:, :, :].rearrange("p a b -> p (a b)")
                else:
                    ps = drP.tile([P, 2 * CH], F32, tag="dr")
                for half in range(2):
                    o_t = 2 * ot2 + half
                    for g in range(2):
                        nc.tensor.matmul(
                            ps[:, half * CH : (half + 1) * CH],
                            lhsT=woT2_sb[:, g, o_t * P : (o_t + 1) * P],
                            rhs=OT128[:, g, sl],
                            start=(g == 0),
                            stop=(g == 1),
                        )
                    yield
                copy_via(ot2 + 1, pstage[:, 2 * ot2 : 2 * ot2 + 2, :], ps, act_every=4)
                if ot2 in (3, 7):
                    hb = 8 * (ot2 // 4)
                    nc.sync.dma_start(
                        partT_r[:, c, hb : hb + 8, :], pstage[:, hb : hb + 8, :]
                    )
                yield
            emit_rs(c)

        def emit_rs(c):
            """ReduceScatter chunk c. The y output DMAs are all deferred to
            the end of the program: a y DMA waits ~21us on its collective,
            and on the in-order SP queue that wait would head-block the next
            chunk's partT DMAs, serializing every collective."""
            nc.gpsimd.collective_compute(
                "ReduceScatter",
                mybir.AluOpType.add,
                replica_groups=[list(range(N_CORES))],
                ins=[partT_dram[c, :, :]],
                outs=[ysT_dram[c, :, :]],
            )

        def emit_attention(c, filler):
            """Attention for the 4 local heads of q-chunk c, exp grouped over
            2 adjacent k-blocks; pulls from filler (drain of chunk c-1)
            between ST and PV so PE never idles while Exp runs."""
            nj = 4 * c + 4
            stg = nrm.tile([HD + 1, NH_LOC, CH], BF16, tag="stg")
            sl = slice(c * CH, (c + 1) * CH)
            for h in (0, 2, 1, 3):
                lo = HD * (h % 2)
                qrhs = qAll[lo : lo + HD, h // 2, sl]
                ot = otP.tile([P, CH], F32, tag="ot")

                def do_st(j2):
                    st = stP.tile([P, 2, CH], F32, tag="st")
                    for i in range(2):
                        nc.tensor.matmul(
                            st[:, i, :],
                            lhsT=kT2[lo : lo + HD, (2 * j2 + i) * P : (2 * j2 + i + 1) * P],
                            rhs=qrhs,
                            start=True,
                            stop=True,
                        )
                    return st

                def do_rest(st, j2):
                    pt = pt_p.tile([P, 2, CH], BF16, tag="pt")
                    nc.scalar.activation(
                        pt, st, mybir.ActivationFunctionType.Exp, scale=SCALE
                    )
                    if 2 * j2 >= 4 * c:
                        # diagonal group: per-block column trim. Block at
                        # diag offset d contributes nothing to cols < 128d;
                        # only the 128-wide triangle at [128d, 128d+128)
                        # needs masking; cols >= 128(d+1) are fully valid.
                        for i in range(2):
                            j = 2 * j2 + i
                            d = j - 4 * c
                            nc.vector.tensor_mul(
                                pt[:, i, P * d : P * (d + 1)],
                                pt[:, i, P * d : P * (d + 1)],
                                tri_sb,
                            )
                            nc.tensor.matmul(
                                ot[0 : HD + 1, P * d : CH],
                                lhsT=v_aug[:, j, :],
                                rhs=pt[:, i, P * d : CH],
                                start=(j == 0),
                                stop=(j == nj - 1),
                                skip_group_check=True,
                            )
                    else:
                        for i in range(2):
                            j = 2 * j2 + i
                            nc.tensor.matmul(
                                ot[0 : HD + 1, :],
                                lhsT=v_aug[:, j, :],
                                rhs=pt[:, i, :],
                                start=(j == 0),
                                stop=False,
                                skip_group_check=True,
                            )

                st_cur = do_st(0)
                for j2 in range(nj // 2):
                    st_next = do_st(j2 + 1) if j2 + 1 < nj // 2 else None
                    next(filler, None)
                    do_rest(st_cur, j2)
                    next(filler, None)
                    next(filler, None)
                    st_cur = st_next

                # stage unnormalized O^T + denominator row (bf16)
                copy_via(h, stg[:, h, :], ot[0 : HD + 1, :])
            return stg

        def emit_norm(c, stg):
            """Softmax normalization for chunk c (batched over 4 heads)."""
            sl = slice(c * CH, (c + 1) * CH)
            denT = nrm.tile([1, NH_LOC, CH], BF16, tag="den")
            nc.gpsimd.dma_start(denT[0:1, :, :], stg[HD : HD + 1, :, :])
            rcpT = nrm.tile([1, NH_LOC, CH], BF16, tag="rcp")
            with nc.allow_low_precision("softmax denom in bf16 is fine"):
                nc.vector.reciprocal(rcpT, denT)
            yield
            for h in range(NH_LOC):
                bc = otP.tile([P, CH], F32, tag="ot")
                nc.tensor.matmul(
                    bc[0:HD, :], lhsT=ones64, rhs=rcpT[0:1, h, :],
                    start=True, stop=True,
                )
                if h % 2 == 0:
                    nc.vector.tensor_mul(
                        OT128[0:HD, h // 2, sl], stg[0:HD, h, :], bc[0:HD, :]
                    )
                else:
                    oddt = nrm.tile([HD, CH], BF16, tag="oddt")
                    nc.vector.tensor_mul(oddt, stg[0:HD, h, :], bc[0:HD, :])
                    nc.gpsimd.dma_start(OT128[HD:P, h // 2, sl], oddt)
                yield

        # ---- prologue: chunk 0's proj/rope dense; everything else (proj of
        # later chunks, norm + partial-Wo drain + ReduceScatter of earlier
        # chunks) rides attention's PE bubbles via the filler generators.
        for _ in emit_proj_rope(0):
            pass
        filler = emit_proj_rope(1)
        for c in range(NCH):
            stg = emit_attention(c, filler)
            for _ in filler:
                pass
            gens = []
            if c + 2 < NCH:
                gens.append(emit_proj_rope(c + 2))
            gens.append(emit_norm(c, stg))
            gens.append(drain_gen(c, final=(c == NCH - 1)))
            filler = itertools.chain(*gens)
        # final norm+drain runs dense (its ReduceScatter is emitted inline)
        for _ in filler:
            pass
        y_r = y_d.rearrange("c (r p) t -> p c r t", p=P)
        ys_r = ysT_dram.rearrange("c (r p) t -> p c r t", p=P)
        for c in range(NCH):
            nc.sync.dma_start(y_r[:, c, :, :], ys_r[:, c, :, :])


# revision 57
# speedup vs baseline: 2.0674x; 1.0227x over previous
"""GQA attention with LoRA-Q, tensor-parallel over 8 TRN2 cores.

Sharding (per core i of 8):
  - Q heads 4i..4i+3 (256 q-dims) and KV head i (GQA: repeat_interleave maps
    q heads [4i,4i+4) exactly onto kv head i).
  - Wq (with LoRA folded: Wq_eff = Wq + lora_B @ lora_A), Wk, Wv row-sharded;
    Wo row-parallel on its input (head) dim: each core computes the full-width
    partial y^T from its own 256 head-dims; four per-T-chunk ReduceScatter(add)
    ops (overlapped with attention of later chunks) leave each core its
    256-feature slice of y^T (transposed + concatenated on the host).

Structure is a single fused c-outer pipeline over the four 512-token chunks:
  proj c -> rope c -> attention (head pairs, shared kv) c -> norm c ->
  partial-Wo drain c -> ReduceScatter #c
so the collective and DMA traffic hide under attention of later chunks.

All matmuls in bf16 with fp32 PSUM accumulation; softmax without max
subtraction (scores are bounded: |S/8| <= ~7), denominator fused into the
PV matmul via an appended ones-column on V. RoPE's rotate-half is a signed
permutation matmul on PE (no partition-shuffle DMAs).
"""

import numpy as np
import ml_dtypes

import concourse.bass as bass
import concourse.mybir as mybir
import concourse.tile as tile
from concourse import bacc
from concourse.bass_utils import run_bass_kernel_spmd

BF16 = mybir.dt.bfloat16
F32 = mybir.dt.float32
FP8 = mybir.dt.float8e4

N_CORES = 8
T = 2048
D = 2048
HD = 64          # head dim
NH = 32          # total q heads
NKV = 8          # total kv heads
NH_LOC = NH // N_CORES       # 4 q heads per core
QW = NH_LOC * HD             # 256 q dims per core
P = 128
KT = D // P                  # 16 contraction tiles
CH = 512                     # T-chunk
NCH = T // CH                # 4 chunks
NJ = T // P                  # 16 k-blocks
SCALE = 1.0 / 8.0            # 1/sqrt(64)


def build_bass():
    nc = bacc.Bacc(None, num_devices=N_CORES)

    # I/O
    xT_d = nc.dram_tensor("xT", [D, T], BF16, kind="ExternalInput")
    w_d = nc.dram_tensor("w_all", [D, QW + 2 * HD], BF16, kind="ExternalInput")
    woT_d = nc.dram_tensor("woT", [QW, D], BF16, kind="ExternalInput")
    cos2_d = nc.dram_tensor("cos2", [P, T], BF16, kind="ExternalInput")
    sin2_d = nc.dram_tensor("sin2", [P, T], BF16, kind="ExternalInput")
    mask2_d = nc.dram_tensor("dmask2", [P, 4, 2 * CH], BF16, kind="ExternalInput")
    perm_d = nc.dram_tensor("perm", [P, P], BF16, kind="ExternalInput")
    y_d = nc.dram_tensor("y", [NCH, QW, CH], BF16, kind="ExternalOutput")

    with tile.TileContext(nc, num_cores=N_CORES) as tc:
        _body(nc, tc, xT_d, w_d, woT_d, cos2_d, sin2_d, mask2_d, perm_d, y_d)
    nc.compile()
    return nc


def _body(nc, tc, xT_d, w_d, woT_d, cos2_d, sin2_d, mask2_d, perm_d, y_d):
    import contextlib
    import itertools

    ctx = contextlib.ExitStack()
    with ctx:
        consts = ctx.enter_context(tc.tile_pool(name="consts", bufs=1))
        big = ctx.enter_context(tc.tile_pool(name="big", bufs=1))
        work = ctx.enter_context(tc.tile_pool(name="work", bufs=1))
        rp = ctx.enter_context(tc.tile_pool(name="rp", bufs=2))
        pt_p = ctx.enter_context(tc.tile_pool(name="pt_p", bufs=3))
        nrm = ctx.enter_context(tc.tile_pool(name="nrm", bufs=2))
        pst = ctx.enter_context(tc.tile_pool(name="pst", bufs=2))
        stP = ctx.enter_context(tc.tile_pool(name="stP", bufs=2, space="PSUM"))
        otP = ctx.enter_context(tc.tile_pool(name="otP", bufs=2, space="PSUM"))
        drP = ctx.enter_context(tc.tile_pool(name="drP", bufs=1, space="PSUM"))
        dram = ctx.enter_context(tc.tile_pool(name="dram", bufs=1, space="DRAM"))

        # ---- constant tiles (few big DMAs; the DMA path serializes per
        # dma_start, so count matters more than size)
        w_sb = consts.tile([P, KT, QW + 2 * HD], BF16)
        w_r = w_d.rearrange("(kt p) m -> p kt m", p=P)
        for hf in range(2):
            nc.sync.dma_start(w_sb[:, 8 * hf : 8 * hf + 8, :], w_r[:, 8 * hf : 8 * hf + 8, :])
        xT_sb = big.tile([P, KT, T], BF16, tag="big", name="xT_sb")
        xT_r = xT_d.rearrange("(kt p) t -> p kt t", p=P)
        nc.sync.dma_start(xT_sb[:, 0:8, 0:CH], xT_r[:, 0:8, 0:CH])
        nc.sync.dma_start(xT_sb[:, 8:KT, 0:CH], xT_r[:, 8:KT, 0:CH])
        cos2_sb = consts.tile([P, T], BF16)
        nc.sync.dma_start(cos2_sb, cos2_d[:])
        sin2_sb = consts.tile([P, T], BF16)
        nc.sync.dma_start(sin2_sb, sin2_d[:])
        perm_sb = consts.tile([P, P], BF16)
        nc.sync.dma_start(perm_sb, perm_d[:])
        nc.sync.dma_start(xT_sb[:, :, CH : 2 * CH], xT_r[:, :, CH : 2 * CH])
        tri_sb = consts.tile([P, P], BF16)
        nc.sync.dma_start(tri_sb, mask2_d[0:P, 0, 0:P])
        woT2_sb = consts.tile([P, 2, D], BF16)
        woT2_r = woT_d.rearrange("(g p) o -> p g o", p=P)
        nc.sync.dma_start(woT2_sb, woT2_r[:, :, :])
        nc.sync.dma_start(xT_sb[:, :, 2 * CH : 3 * CH], xT_r[:, :, 2 * CH : 3 * CH])
        nc.sync.dma_start(xT_sb[:, :, 3 * CH : 4 * CH], xT_r[:, :, 3 * CH : 4 * CH])

        ones64 = consts.tile([1, HD], BF16)
        nc.vector.memset(ones64, 1.0)

        # v with ones column appended: [tk(P), j, HD+1]
        v_aug = work.tile([P, NJ, HD + 1], BF16)
        nc.vector.memset(v_aug[:, :, HD : HD + 1], 1.0)

        # k^T duplicated on both partition halves so odd heads (whose rope
        # output lives at partitions 64:128) can matmul base-aligned
        kT2 = work.tile([P, T], BF16)
        # rope'd q pairs for all chunks: qAll[64*(h%2)+d, h//2, t]
        qAll = work.tile([P, 2, T], BF16)
        # O^T repacked to 128 partitions: OT128[64*(h%2)+d, h//2, t]
        # (partition p of pair g is local head-dim 128*g+p, matching woT2)
        OT128 = work.tile([P, 2, T], BF16)

        # chunk-major so each chunk's collective sees contiguous DRAM
        partT_dram = dram.tile([NCH, D, CH], BF16)
        partT_r = partT_dram.rearrange("c (ot p) t -> p c ot t", p=P)
        ysT_dram = dram.tile([NCH, QW, CH], BF16)

        def copy_via(idx, dst, src, act_every=2):
            if idx % act_every == 0:
                nc.scalar.copy(dst, src)
            else:
                nc.vector.tensor_copy(dst, src)

        def emit_proj_rope(c):
            """QKV projection + RoPE + t-partitioned V for chunk c, as a
            generator so chunks >= 1 can ride attention's PE bubbles."""
            sl = slice(c * CH, (c + 1) * CH)
            projT = rp.tile([P, 3, CH], BF16, tag="projT")
            for m in (2, 0, 1):
                ps = drP.tile([P, 2 * CH], F32, tag="dr")
                for kt in range(KT):
                    nc.tensor.matmul(
                        ps[:, 0:CH],
                        lhsT=w_sb[:, kt, m * P : (m + 1) * P],
                        rhs=xT_sb[:, kt, sl],
                        start=(kt == 0),
                        stop=(kt == KT - 1),
                    )
                    if kt % 8 == 7:
                        yield
                nc.vector.tensor_copy(projT[:, m, :], ps[:, 0:CH])
                yield
                if m == 2:
                    # RoPE k -> kT2[0:64, sl] + dup to [64:128]
                    ksh = drP.tile([P, 2 * CH], F32, tag="dr")
                    nc.tensor.matmul(
                        ksh[0:HD, 0:CH], lhsT=perm_sb[0:HD, 0:HD],
                        rhs=projT[0:HD, 2, :], start=True, stop=True,
                    )
                    kt2 = rp.tile([P, CH], BF16, tag="t2")
                    nc.vector.tensor_mul(kt2[0:HD, :], ksh[0:HD, 0:CH], sin2_sb[0:HD, sl])
                    veng = nc.vector if c == 0 else nc.gpsimd
                    kt1 = rp.tile([P, CH], BF16, tag="t1")
                    veng.tensor_mul(kt1[0:HD, :], projT[0:HD, 2, :], cos2_sb[0:HD, sl])
                    veng.tensor_add(kT2[0:HD, sl], kt1[0:HD, :], kt2[0:HD, :])
                    nc.gpsimd.dma_start(kT2[HD:P, sl], kT2[0:HD, sl])
                    yield
                    # v in t-partitioned layout (free dim 64 -> 27ns/matmul)
                    for tb in range(4):
                        pv = drP.tile([P, 2 * CH], F32, tag="dr")
                        for kt in range(KT):
                            nc.tensor.matmul(
                                pv[:, 0:HD],
                                lhsT=xT_sb[:, kt, (4 * c + tb) * P : (4 * c + tb + 1) * P],
                                rhs=w_sb[:, kt, QW + HD : QW + 2 * HD],
                                start=(kt == 0),
                                stop=(kt == KT - 1),
                            )
                        nc.vector.tensor_copy(v_aug[:, 4 * c + tb, 0:HD], pv[:, 0:HD])
                        yield
                else:
                    s = m
                    qsh = drP.tile([P, 2 * CH], F32, tag="dr")
                    nc.tensor.matmul(
                        qsh[:, 0:CH], lhsT=perm_sb, rhs=projT[:, s, :],
                        start=True, stop=True,
                    )
                    yield
                    t2 = rp.tile([P, CH], BF16, tag="t2")
                    nc.vector.tensor_mul(t2, qsh[:, 0:CH], sin2_sb[:, sl])
                    veng = nc.vector if c == 0 else nc.gpsimd
                    t1 = rp.tile([P, CH], BF16, tag="t1")
                    veng.tensor_mul(t1, projT[:, s, :], cos2_sb[:, sl])
                    veng.tensor_add(qAll[:, s, sl], t1, t2)
                    yield

        def drain_gen(c, final=False):
            """Chunk c's partial-Wo drain + partT DMAs + ReduceScatter.
            Interleaved into attention (c+1)'s j-loop as PE filler; the final
            drain double-buffers by alternating PSUM between drP and the
            idle stP."""
            sl = slice(c * CH, (c + 1) * CH)
            pstage = pst.tile([P, KT, CH], BF16, tag="pst")
            for ot2 in range(8):
                if final and ot2 % 2 == 1:
                    ps = stP.tile([P, 2, CH], F32, tag="st")
                    ps = ps[:, :, :].rearrange("p a b -> p (a b)")
                else:
                    ps = drP.tile([P, 2 * CH], F32, tag="dr")
                for half in range(2):
                    o_t = 2 * ot2 + half
                    for g in range(2):
                        nc.tensor.matmul(
                            ps[:, half * CH : (half + 1) * CH],
                            lhsT=woT2_sb[:, g, o_t * P : (o_t + 1) * P],
                            rhs=OT128[:, g, sl],
                            start=(g == 0),
                            stop=(g == 1),
                        )
                    yield
                copy_via(ot2 + 1, pstage[:, 2 * ot2 : 2 * ot2 + 2, :], ps, act_every=4)
                if final and ot2 % 2 == 1:
                    hb = 4 * (ot2 // 2)
                    nc.sync.dma_start(
                        partT_r[:, c, hb : hb + 4, :], pstage[:, hb : hb + 4, :]
                    )
                elif not final and ot2 in (3, 7):
                    hb = 8 * (ot2 // 4)
                    nc.sync.dma_start(
                        partT_r[:, c, hb : hb + 8, :], pstage[:, hb : hb + 8, :]
                    )
                yield
            emit_rs(c)

        def emit_rs(c):
            """ReduceScatter chunk c. The y output DMAs are all deferred to
            the end of the program: a y DMA waits ~21us on its collective,
            and on the in-order SP queue that wait would head-block the next
            chunk's partT DMAs, serializing every collective."""
            nc.gpsimd.collective_compute(
                "ReduceScatter",
                mybir.AluOpType.add,
                replica_groups=[list(range(N_CORES))],
                ins=[partT_dram[c, :, :]],
                outs=[ysT_dram[c, :, :]],
            )

        def emit_attention(c, filler):
            """Attention for the 4 local heads of q-chunk c, exp grouped over
            2 adjacent k-blocks; pulls from filler (drain of chunk c-1)
            between ST and PV so PE never idles while Exp runs."""
            nj = 4 * c + 4
            stg = nrm.tile([HD + 1, NH_LOC, CH], BF16, tag="stg")
            sl = slice(c * CH, (c + 1) * CH)
            for h in (0, 2, 1, 3):
                lo = HD * (h % 2)
                qrhs = qAll[lo : lo + HD, h // 2, sl]
                ot = otP.tile([P, CH], F32, tag="ot")

                def do_st(j2):
                    st = stP.tile([P, 2, CH], F32, tag="st")
                    for i in range(2):
                        nc.tensor.matmul(
                            st[:, i, :],
                            lhsT=kT2[lo : lo + HD, (2 * j2 + i) * P : (2 * j2 + i + 1) * P],
                            rhs=qrhs,
                            start=True,
                            stop=True,
                        )
                    return st

                def do_rest(st, j2):
                    pt = pt_p.tile([P, 2, CH], BF16, tag="pt")
                    nc.scalar.activation(
                        pt, st, mybir.ActivationFunctionType.Exp, scale=SCALE
                    )
                    if 2 * j2 >= 4 * c:
                        # diagonal group: per-block column trim. Block at
                        # diag offset d contributes nothing to cols < 128d;
                        # only the 128-wide triangle at [128d, 128d+128)
                        # needs masking; cols >= 128(d+1) are fully valid.
                        for i in range(2):
                            j = 2 * j2 + i
                            d = j - 4 * c
                            nc.vector.tensor_mul(
                                pt[:, i, P * d : P * (d + 1)],
                                pt[:, i, P * d : P * (d + 1)],
                                tri_sb,
                            )
                            nc.tensor.matmul(
                                ot[0 : HD + 1, P * d : CH],
                                lhsT=v_aug[:, j, :],
                                rhs=pt[:, i, P * d : CH],
                                start=(j == 0),
                                stop=(j == nj - 1),
                                skip_group_check=True,
                            )
                    else:
                        for i in range(2):
                            j = 2 * j2 + i
                            nc.tensor.matmul(
                                ot[0 : HD + 1, :],
                                lhsT=v_aug[:, j, :],
                                rhs=pt[:, i, :],
                                start=(j == 0),
                                stop=False,
                                skip_group_check=True,
                            )

                st_cur = do_st(0)
                for j2 in range(nj // 2):
                    st_next = do_st(j2 + 1) if j2 + 1 < nj // 2 else None
                    next(filler, None)
                    do_rest(st_cur, j2)
                    next(filler, None)
                    next(filler, None)
                    st_cur = st_next

                # stage unnormalized O^T + denominator row (bf16)
                copy_via(h, stg[:, h, :], ot[0 : HD + 1, :])
            return stg

        def emit_norm(c, stg):
            """Softmax normalization for chunk c (batched over 4 heads)."""
            sl = slice(c * CH, (c + 1) * CH)
            denT = nrm.tile([1, NH_LOC, CH], BF16, tag="den")
            nc.gpsimd.dma_start(denT[0:1, :, :], stg[HD : HD + 1, :, :])
            rcpT = nrm.tile([1, NH_LOC, CH], BF16, tag="rcp")
            with nc.allow_low_precision("softmax denom in bf16 is fine"):
                nc.vector.reciprocal(rcpT[0:1, 0:2, :], denT[0:1, 0:2, :])
                nc.vector.reciprocal(rcpT[0:1, 2:4, :], denT[0:1, 2:4, :])
            yield
            for h in range(NH_LOC):
                bc = otP.tile([P, CH], F32, tag="ot")
                nc.tensor.matmul(
                    bc[0:HD, :], lhsT=ones64, rhs=rcpT[0:1, h, :],
                    start=True, stop=True,
                )
                if h % 2 == 0:
                    nc.vector.tensor_mul(
                        OT128[0:HD, h // 2, sl], stg[0:HD, h, :], bc[0:HD, :]
                    )
                else:
                    oddt = nrm.tile([HD, CH], BF16, tag="oddt")
                    nc.vector.tensor_mul(oddt, stg[0:HD, h, :], bc[0:HD, :])
                    nc.gpsimd.dma_start(OT128[HD:P, h // 2, sl], oddt)
                yield

        # ---- prologue: chunk 0's proj/rope dense; everything else (proj of
        # later chunks, norm + partial-Wo drain + ReduceScatter of earlier
        # chunks) rides attention's PE bubbles via the filler generators.
        for _ in emit_proj_rope(0):
            pass
        filler = emit_proj_rope(1)
        for c in range(NCH):
            stg = emit_attention(c, filler)
            for _ in filler:
                pass
            gens = []
            if c + 2 < NCH:
                gens.append(emit_proj_rope(c + 2))
            gens.append(emit_norm(c, stg))
            gens.append(drain_gen(c, final=(c == NCH - 1)))
            filler = itertools.chain(*gens)
        # final norm+drain runs dense (its ReduceScatter is emitted inline)
        for _ in filler:
            pass
        y_r = y_d.rearrange("c (r p) t -> p c r t", p=P)
        ys_r = ysT_dram.rearrange("c (r p) t -> p c r t", p=P)
        for c in range(NCH):
            nc.sync.dma_start(y_r[:, c, :, :], ys_r[:, c, :, :])


# revision 60
# speedup vs baseline: 2.1147x; 1.0229x over previous
"""GQA attention with LoRA-Q, tensor-parallel over 8 TRN2 cores.

Sharding (per core i of 8):
  - Q heads 4i..4i+3 (256 q-dims) and KV head i (GQA: repeat_interleave maps
    q heads [4i,4i+4) exactly onto kv head i).
  - Wq (with LoRA folded: Wq_eff = Wq + lora_B @ lora_A), Wk, Wv row-sharded;
    Wo row-parallel on its input (head) dim: each core computes the full-width
    partial y^T from its own 256 head-dims; four per-T-chunk ReduceScatter(add)
    ops (overlapped with attention of later chunks) leave each core its
    256-feature slice of y^T (transposed + concatenated on the host).

Structure is a single fused c-outer pipeline over the four 512-token chunks:
  proj c -> rope c -> attention (head pairs, shared kv) c -> norm c ->
  partial-Wo drain c -> ReduceScatter #c
so the collective and DMA traffic hide under attention of later chunks.

All matmuls in bf16 with fp32 PSUM accumulation; softmax without max
subtraction (scores are bounded: |S/8| <= ~7), denominator fused into the
PV matmul via an appended ones-column on V. RoPE's rotate-half is a signed
permutation matmul on PE (no partition-shuffle DMAs).
"""

import numpy as np
import ml_dtypes

import concourse.bass as bass
import concourse.mybir as mybir
import concourse.tile as tile
from concourse import bacc
from concourse.bass_utils import run_bass_kernel_spmd

BF16 = mybir.dt.bfloat16
F32 = mybir.dt.float32
FP8 = mybir.dt.float8e4

N_CORES = 8
T = 2048
D = 2048
HD = 64          # head dim
NH = 32          # total q heads
NKV = 8          # total kv heads
NH_LOC = NH // N_CORES       # 4 q heads per core
QW = NH_LOC * HD             # 256 q dims per core
P = 128
KT = D // P                  # 16 contraction tiles
CH = 512                     # T-chunk
NCH = T // CH                # 4 chunks
NJ = T // P                  # 16 k-blocks
SCALE = 1.0 / 8.0            # 1/sqrt(64)


def build_bass():
    nc = bacc.Bacc(None, num_devices=N_CORES)

    # I/O
    xT_d = nc.dram_tensor("xT", [D, T], BF16, kind="ExternalInput")
    w_d = nc.dram_tensor("w_all", [D, QW + 2 * HD], BF16, kind="ExternalInput")
    woT_d = nc.dram_tensor("woT", [QW, D], BF16, kind="ExternalInput")
    cos2_d = nc.dram_tensor("cos2", [P, T], BF16, kind="ExternalInput")
    sin2_d = nc.dram_tensor("sin2", [P, T], BF16, kind="ExternalInput")
    mask2_d = nc.dram_tensor("dmask2", [P, 4, 2 * CH], BF16, kind="ExternalInput")
    perm_d = nc.dram_tensor("perm", [P, P], BF16, kind="ExternalInput")
    y_d = nc.dram_tensor("y", [NCH, QW, CH], BF16, kind="ExternalOutput")

    with tile.TileContext(nc, num_cores=N_CORES) as tc:
        _body(nc, tc, xT_d, w_d, woT_d, cos2_d, sin2_d, mask2_d, perm_d, y_d)
    nc.compile()
    return nc


def _body(nc, tc, xT_d, w_d, woT_d, cos2_d, sin2_d, mask2_d, perm_d, y_d):
    import contextlib
    import itertools

    ctx = contextlib.ExitStack()
    with ctx:
        consts = ctx.enter_context(tc.tile_pool(name="consts", bufs=1))
        big = ctx.enter_context(tc.tile_pool(name="big", bufs=1))
        work = ctx.enter_context(tc.tile_pool(name="work", bufs=1))
        rp = ctx.enter_context(tc.tile_pool(name="rp", bufs=2))
        pt_p = ctx.enter_context(tc.tile_pool(name="pt_p", bufs=3))
        nrm = ctx.enter_context(tc.tile_pool(name="nrm", bufs=2))
        pst = ctx.enter_context(tc.tile_pool(name="pst", bufs=2))
        stP = ctx.enter_context(tc.tile_pool(name="stP", bufs=2, space="PSUM"))
        otP = ctx.enter_context(tc.tile_pool(name="otP", bufs=2, space="PSUM"))
        drP = ctx.enter_context(tc.tile_pool(name="drP", bufs=1, space="PSUM"))
        dram = ctx.enter_context(tc.tile_pool(name="dram", bufs=1, space="DRAM"))

        # ---- constant tiles (few big DMAs; the DMA path serializes per
        # dma_start, so count matters more than size)
        w_sb = consts.tile([P, KT, QW + 2 * HD], BF16)
        w_r = w_d.rearrange("(kt p) m -> p kt m", p=P)
        xT_sb = big.tile([P, KT, T], BF16, tag="big", name="xT_sb")
        xT_r = xT_d.rearrange("(kt p) t -> p kt t", p=P)
        nc.sync.dma_start(w_sb[:, 0:8, :], w_r[:, 0:8, :])
        nc.sync.dma_start(xT_sb[:, 0:8, 0:CH], xT_r[:, 0:8, 0:CH])
        nc.sync.dma_start(w_sb[:, 8:KT, :], w_r[:, 8:KT, :])
        nc.sync.dma_start(xT_sb[:, 8:KT, 0:CH], xT_r[:, 8:KT, 0:CH])
        cos2_sb = consts.tile([P, T], BF16)
        nc.sync.dma_start(cos2_sb, cos2_d[:])
        sin2_sb = consts.tile([P, T], BF16)
        nc.sync.dma_start(sin2_sb, sin2_d[:])
        perm_sb = consts.tile([P, P], BF16)
        nc.sync.dma_start(perm_sb, perm_d[:])
        nc.sync.dma_start(xT_sb[:, :, CH : 2 * CH], xT_r[:, :, CH : 2 * CH])
        tri_sb = consts.tile([P, P], BF16)
        nc.sync.dma_start(tri_sb, mask2_d[0:P, 0, 0:P])
        woT2_sb = consts.tile([P, 2, D], BF16)
        woT2_r = woT_d.rearrange("(g p) o -> p g o", p=P)
        nc.sync.dma_start(woT2_sb, woT2_r[:, :, :])
        nc.sync.dma_start(xT_sb[:, :, 2 * CH : 3 * CH], xT_r[:, :, 2 * CH : 3 * CH])
        nc.sync.dma_start(xT_sb[:, :, 3 * CH : 4 * CH], xT_r[:, :, 3 * CH : 4 * CH])

        ones64 = consts.tile([1, HD], BF16)
        nc.vector.memset(ones64, 1.0)

        # v with ones column appended: [tk(P), j, HD+1]
        v_aug = work.tile([P, NJ, HD + 1], BF16)
        nc.vector.memset(v_aug[:, :, HD : HD + 1], 1.0)

        # k^T duplicated on both partition halves so odd heads (whose rope
        # output lives at partitions 64:128) can matmul base-aligned
        kT2 = work.tile([P, T], BF16)
        # rope'd q pairs for all chunks: qAll[64*(h%2)+d, h//2, t]
        qAll = work.tile([P, 2, T], BF16)
        # O^T repacked to 128 partitions: OT128[64*(h%2)+d, h//2, t]
        # (partition p of pair g is local head-dim 128*g+p, matching woT2)
        OT128 = work.tile([P, 2, T], BF16)

        # chunk-major so each chunk's collective sees contiguous DRAM
        partT_dram = dram.tile([NCH, D, CH], BF16)
        partT_r = partT_dram.rearrange("c (ot p) t -> p c ot t", p=P)
        ysT_dram = dram.tile([NCH, QW, CH], BF16)

        def copy_via(idx, dst, src, act_every=2):
            if idx % act_every == 0:
                nc.scalar.copy(dst, src)
            else:
                nc.vector.tensor_copy(dst, src)

        def emit_proj_rope(c):
            """QKV projection + RoPE + t-partitioned V for chunk c, as a
            generator so chunks >= 1 can ride attention's PE bubbles."""
            sl = slice(c * CH, (c + 1) * CH)
            projT = rp.tile([P, 3, CH], BF16, tag="projT")
            for m in (2, 0, 1):
                ps = drP.tile([P, 2 * CH], F32, tag="dr")
                for kt in range(KT):
                    nc.tensor.matmul(
                        ps[:, 0:CH],
                        lhsT=w_sb[:, kt, m * P : (m + 1) * P],
                        rhs=xT_sb[:, kt, sl],
                        start=(kt == 0),
                        stop=(kt == KT - 1),
                    )
                    if kt % 8 == 7:
                        yield
                nc.vector.tensor_copy(projT[:, m, :], ps[:, 0:CH])
                yield
                if m == 2:
                    # RoPE k -> kT2[0:64, sl] + dup to [64:128]
                    ksh = drP.tile([P, 2 * CH], F32, tag="dr")
                    nc.tensor.matmul(
                        ksh[0:HD, 0:CH], lhsT=perm_sb[0:HD, 0:HD],
                        rhs=projT[0:HD, 2, :], start=True, stop=True,
                    )
                    kt2 = rp.tile([P, CH], BF16, tag="t2")
                    nc.vector.tensor_mul(kt2[0:HD, :], ksh[0:HD, 0:CH], sin2_sb[0:HD, sl])
                    veng = nc.vector if c == 0 else nc.gpsimd
                    kt1 = rp.tile([P, CH], BF16, tag="t1")
                    veng.tensor_mul(kt1[0:HD, :], projT[0:HD, 2, :], cos2_sb[0:HD, sl])
                    veng.tensor_add(kT2[0:HD, sl], kt1[0:HD, :], kt2[0:HD, :])
                    nc.gpsimd.dma_start(kT2[HD:P, sl], kT2[0:HD, sl])
                    yield
                    # v in t-partitioned layout (free dim 64 -> 27ns/matmul)
                    for tb in range(4):
                        pv = drP.tile([P, 2 * CH], F32, tag="dr")
                        for kt in range(KT):
                            nc.tensor.matmul(
                                pv[:, 0:HD],
                                lhsT=xT_sb[:, kt, (4 * c + tb) * P : (4 * c + tb + 1) * P],
                                rhs=w_sb[:, kt, QW + HD : QW + 2 * HD],
                                start=(kt == 0),
                                stop=(kt == KT - 1),
                            )
                        nc.vector.tensor_copy(v_aug[:, 4 * c + tb, 0:HD], pv[:, 0:HD])
                        yield
                else:
                    s = m
                    qsh = drP.tile([P, 2 * CH], F32, tag="dr")
                    nc.tensor.matmul(
                        qsh[:, 0:CH], lhsT=perm_sb, rhs=projT[:, s, :],
                        start=True, stop=True,
                    )
                    yield
                    t2 = rp.tile([P, CH], BF16, tag="t2")
                    nc.vector.tensor_mul(t2, qsh[:, 0:CH], sin2_sb[:, sl])
                    veng = nc.vector if c == 0 else nc.gpsimd
                    t1 = rp.tile([P, CH], BF16, tag="t1")
                    veng.tensor_mul(t1, projT[:, s, :], cos2_sb[:, sl])
                    veng.tensor_add(qAll[:, s, sl], t1, t2)
                    yield

        def drain_gen(c, final=False):
            """Chunk c's partial-Wo drain + partT DMAs + ReduceScatter.
            Interleaved into attention (c+1)'s j-loop as PE filler; the final
            drain double-buffers by alternating PSUM between drP and the
            idle stP."""
            sl = slice(c * CH, (c + 1) * CH)
            pstage = pst.tile([P, KT, CH], BF16, tag="pst")
            for ot2 in range(8):
                if final and ot2 % 2 == 1:
                    ps = stP.tile([P, 2, CH], F32, tag="st")
                    ps = ps[:, :, :].rearrange("p a b -> p (a b)")
                else:
                    ps = drP.tile([P, 2 * CH], F32, tag="dr")
                for half in range(2):
                    o_t = 2 * ot2 + half
                    for g in range(2):
                        nc.tensor.matmul(
                            ps[:, half * CH : (half + 1) * CH],
                            lhsT=woT2_sb[:, g, o_t * P : (o_t + 1) * P],
                            rhs=OT128[:, g, sl],
                            start=(g == 0),
                            stop=(g == 1),
                        )
                    yield
                copy_via(ot2 + 1, pstage[:, 2 * ot2 : 2 * ot2 + 2, :],
                         ps, act_every=2 if final else 4)
                if final and ot2 % 2 == 1:
                    hb = 4 * (ot2 // 2)
                    nc.sync.dma_start(
                        partT_r[:, c, hb : hb + 4, :], pstage[:, hb : hb + 4, :]
                    )
                elif not final and ot2 in (3, 7):
                    hb = 8 * (ot2 // 4)
                    nc.sync.dma_start(
                        partT_r[:, c, hb : hb + 8, :], pstage[:, hb : hb + 8, :]
                    )
                yield
            emit_rs(c)

        def emit_rs(c):
            """ReduceScatter chunk c. The y output DMAs are all deferred to
            the end of the program: a y DMA waits ~21us on its collective,
            and on the in-order SP queue that wait would head-block the next
            chunk's partT DMAs, serializing every collective."""
            nc.gpsimd.collective_compute(
                "ReduceScatter",
                mybir.AluOpType.add,
                replica_groups=[list(range(N_CORES))],
                ins=[partT_dram[c, :, :]],
                outs=[ysT_dram[c, :, :]],
            )

        def emit_attention(c, filler):
            """Attention for the 4 local heads of q-chunk c, exp grouped over
            2 adjacent k-blocks; pulls from filler (drain of chunk c-1)
            between ST and PV so PE never idles while Exp runs."""
            nj = 4 * c + 4
            stg = nrm.tile([HD + 1, NH_LOC, CH], BF16, tag="stg")
            sl = slice(c * CH, (c + 1) * CH)
            for h in (0, 2, 1, 3):
                lo = HD * (h % 2)
                qrhs = qAll[lo : lo + HD, h // 2, sl]
                ot = otP.tile([P, CH], F32, tag="ot")

                def do_st(j2):
                    st = stP.tile([P, 2, CH], F32, tag="st")
                    for i in range(2):
                        nc.tensor.matmul(
                            st[:, i, :],
                            lhsT=kT2[lo : lo + HD, (2 * j2 + i) * P : (2 * j2 + i + 1) * P],
                            rhs=qrhs,
                            start=True,
                            stop=True,
                        )
                    return st

                def do_rest(st, j2):
                    pt = pt_p.tile([P, 2, CH], BF16, tag="pt")
                    nc.scalar.activation(
                        pt, st, mybir.ActivationFunctionType.Exp, scale=SCALE
                    )
                    if 2 * j2 >= 4 * c:
                        # diagonal group: per-block column trim. Block at
                        # diag offset d contributes nothing to cols < 128d;
                        # only the 128-wide triangle at [128d, 128d+128)
                        # needs masking; cols >= 128(d+1) are fully valid.
                        for i in range(2):
                            j = 2 * j2 + i
                            d = j - 4 * c
                            nc.vector.tensor_mul(
                                pt[:, i, P * d : P * (d + 1)],
                                pt[:, i, P * d : P * (d + 1)],
                                tri_sb,
                            )
                            nc.tensor.matmul(
                                ot[0 : HD + 1, P * d : CH],
                                lhsT=v_aug[:, j, :],
                                rhs=pt[:, i, P * d : CH],
                                start=(j == 0),
                                stop=(j == nj - 1),
                                skip_group_check=True,
                            )
                    else:
                        for i in range(2):
                            j = 2 * j2 + i
                            nc.tensor.matmul(
                                ot[0 : HD + 1, :],
                                lhsT=v_aug[:, j, :],
                                rhs=pt[:, i, :],
                                start=(j == 0),
                                stop=False,
                                skip_group_check=True,
                            )

                st_cur = do_st(0)
                for j2 in range(nj // 2):
                    st_next = do_st(j2 + 1) if j2 + 1 < nj // 2 else None
                    next(filler, None)
                    do_rest(st_cur, j2)
                    next(filler, None)
                    next(filler, None)
                    st_cur = st_next

                # stage unnormalized O^T + denominator row (bf16)
                copy_via(h, stg[:, h, :], ot[0 : HD + 1, :])
            return stg

        def emit_norm(c, stg):
            """Softmax normalization for chunk c (batched over 4 heads)."""
            sl = slice(c * CH, (c + 1) * CH)
            denT = nrm.tile([1, NH_LOC, CH], BF16, tag="den")
            nc.gpsimd.dma_start(denT[0:1, :, :], stg[HD : HD + 1, :, :])
            rcpT = nrm.tile([1, NH_LOC, CH], BF16, tag="rcp")
            with nc.allow_low_precision("softmax denom in bf16 is fine"):
                nc.vector.reciprocal(rcpT[0:1, 0:2, :], denT[0:1, 0:2, :])
                nc.vector.reciprocal(rcpT[0:1, 2:4, :], denT[0:1, 2:4, :])
            yield
            for h in range(NH_LOC):
                bc = otP.tile([P, CH], F32, tag="ot")
                nc.tensor.matmul(
                    bc[0:HD, :], lhsT=ones64, rhs=rcpT[0:1, h, :],
                    start=True, stop=True,
                )
                if h % 2 == 0:
                    nc.vector.tensor_mul(
                        OT128[0:HD, h // 2, sl], stg[0:HD, h, :], bc[0:HD, :]
                    )
                else:
                    oddt = nrm.tile([HD, CH], BF16, tag="oddt")
                    nc.vector.tensor_mul(oddt, stg[0:HD, h, :], bc[0:HD, :])
                    nc.gpsimd.dma_start(OT128[HD:P, h // 2, sl], oddt)
                yield

        # ---- prologue: chunk 0's proj/rope dense; everything else (proj of
        # later chunks, norm + partial-Wo drain + ReduceScatter of earlier
        # chunks) rides attention's PE bubbles via the filler generators.
        for _ in emit_proj_rope(0):
            pass
        filler = emit_proj_rope(1)
        for c in range(NCH):
            stg = emit_attention(c, filler)
            for _ in filler:
                pass
            gens = []
            if c + 2 < NCH:
                gens.append(emit_proj_rope(c + 2))
            gens.append(emit_norm(c, stg))
            gens.append(drain_gen(c, final=(c == NCH - 1)))
            filler = itertools.chain(*gens)
        # final norm+drain runs dense (its ReduceScatter is emitted inline)
        for _ in filler:
            pass
        y_r = y_d.rearrange("c (r p) t -> p c r t", p=P)
        ys_r = ysT_dram.rearrange("c (r p) t -> p c r t", p=P)
        for c in range(NCH):
            nc.sync.dma_start(y_r[:, c, :, :], ys_r[:, c, :, :])
